# revision 21
# baseline (speedup 1.0000x reference)
"""Trainium2 Bass kernel for a pre-norm transformer block (nn_Block_74766790689102).

v2 strategy (8 NeuronCores, zero-communication SPMD):
  core c handles batch b=c//4, query chunk q=c%4 (512 of 2048 tokens); inputs
  are host-rotated so each core's chunk sits at token positions 0:512 -> one
  identical SPMD program for all 8 cores. Each core redundantly computes K/V
  for its whole batch (attention needs all keys).

  Changes vs v1 (716us):
  - bf16 operands for every matmul (fp32 PSUM accumulation): halves weight
    DMA and enables fast weight loads on the PE.
  - Normalized x is never materialized. QKV matmuls consume raw x^T plus a
    rank-2 correction row ((-mu, sdv) x (col-sums, bias)) folded into the
    contraction, then a per-token rstd scale on the outputs. Kills the
    gpsimd/vector normalize traffic and one full 8MB x reload.
  - K, exp'd scores, attention outputs, res1 all stay in SBUF (no DRAM
    roundtrips).
  - K production for head-pair j+1, scores/exp for j, and PV for j-1 are
    interleaved in one PE stream so the ~134us of scalar-engine exp hides
    behind PE work.
  - Softmax denominators collect into one [16,512] tile; a single
    reciprocal_approx_fast + 8 PE broadcasts replace 16 serial [1,512] DVE
    reciprocals (was 52us).
  - V/proj biases fold into the proj bias on the host (softmax rows sum to 1).
  - w1 is fully resident in SBUF before FC1 starts; w2 streams as bf16.
"""

import os
import sys
import types

import numpy as np
import ml_dtypes

DIM = 1024
HEADS = 16
HD = 64
HIDDEN = 4096
T = 2048          # tokens per batch
CH = 512          # chunk tokens per core
SCALE = HD ** -0.5
EPS = 1e-5
NCT = DIM // 128  # 8 feature tiles
NTC = T // 512    # 4 token chunks
NTT = T // 128    # 16 token tiles
P = 128

_ENV_READY = False
_PROG = None


def _setup_env():
    global _ENV_READY
    if _ENV_READY:
        return
    if "/opt/trn_rl_repo" not in sys.path:
        sys.path.insert(0, "/opt/trn_rl_repo")
    # NTFF profile hook shim (the RL container's antenv lacks axon_hooks).
    try:
        import antenv
        if "antenv.axon_hooks" not in sys.modules:
            mod = types.ModuleType("antenv.axon_hooks")
            mod._hook = None
            mod.set_axon_ntff_profile_hook = lambda h: setattr(mod, "_hook", h)
            mod.get_axon_ntff_profile_hook = lambda: mod._hook
            sys.modules["antenv.axon_hooks"] = mod
            antenv.axon_hooks = mod
        if os.environ.get("BASS_PROFILE"):
            from trn_agent_boot.trn_boot import _ntff_profile_via_ctypes
            sys.modules["antenv.axon_hooks"].set_axon_ntff_profile_hook(
                _ntff_profile_via_ctypes("/opt/axon/libaxon_pjrt.so"))
    except Exception:
        pass
    _ENV_READY = True


def _build_program(zb_proj=False, zb_fc2=False):
    """Build + compile the single-core Bass program (same for all 8 cores).

    zb_proj/zb_fc2: host-verified all-zero proj/fc2 biases let the epilogues
    skip the dead bias adds (general path kept as fallback).
    """
    _setup_env()
    import concourse.bacc as bacc
    import concourse.tile as tile
    import concourse.mybir as mybir
    from concourse.masks import make_identity

    dt = mybir.dt
    AF = mybir.ActivationFunctionType
    ALU = mybir.AluOpType
    f32, bf16 = dt.float32, dt.bfloat16

    nc = bacc.Bacc("TRN2", target_bir_lowering=False, debug=False, num_devices=8)

    # ---- I/O ----
    xtm_d = nc.dram_tensor("xtm_d", [T, DIM], f32, kind="ExternalInput").ap()
    xsb_d = nc.dram_tensor("xsb_d", [T, DIM], bf16, kind="ExternalInput").ap()
    xTb_d = nc.dram_tensor("xTb_d", [DIM, T], bf16, kind="ExternalInput").ap()
    wv_d = nc.dram_tensor("wv_d", [DIM, DIM], bf16, kind="ExternalInput").ap()
    wv1_d = nc.dram_tensor("wv1_d", [1, DIM], bf16, kind="ExternalInput").ap()
    wkq_d = nc.dram_tensor("wkq_d", [NCT, P, NCT, 256], bf16, kind="ExternalInput").ap()
    wkqc_d = nc.dram_tensor("wkqc_d", [2, NCT, 256], bf16, kind="ExternalInput").ap()
    selb_d = nc.dram_tensor("selb_d", [16, NCT, P], bf16, kind="ExternalInput").ap()
    wp_d = nc.dram_tensor("wp_d", [P, NCT, DIM], bf16, kind="ExternalInput").ap()
    bpbc_d = nc.dram_tensor("bpbc_d", [P, DIM], f32, kind="ExternalInput").ap()
    w1h_d = nc.dram_tensor("w1h_d", [HIDDEN // P, P, NCT, P], bf16, kind="ExternalInput").ap()
    b1pp_d = nc.dram_tensor("b1pp_d", [P, HIDDEN // P], f32, kind="ExternalInput").ap()
    w2T_d = nc.dram_tensor("w2T_d", [HIDDEN, DIM], bf16, kind="ExternalInput").ap()
    b2bc_d = nc.dram_tensor("b2bc_d", [P, DIM], f32, kind="ExternalInput").ap()
    out_d = nc.dram_tensor("out_d", [CH, DIM], f32, kind="ExternalOutput").ap()

    with tile.TileContext(nc) as tc:
        with tc.tile_pool(name="cst", bufs=1) as cst, \
             tc.tile_pool(name="resp", bufs=1) as resp:

            # ---------------- constants ----------------
            idf = cst.tile([P, P], f32)
            make_identity(nc, idf[:])
            idb = cst.tile([P, P], bf16)
            nc.vector.tensor_copy(idb[:], idf[:])
            ones1b = cst.tile([1, P], bf16)
            nc.vector.memset(ones1b[:], 1.0)
            eps_t = cst.tile([P, 1], f32)
            nc.vector.memset(eps_t[:], EPS)
            warm = cst.tile([P, 1], f32)
            # head-pair selection matrices for the denominator broadcast
            sel = cst.tile([16, NCT, P], bf16)
            nc.gpsimd.dma_start(sel[:], selb_d[:])
            corr2 = cst.tile([2, T], bf16)     # rows: -mu, sdv (per token)
            rsr = cst.tile([1, T], bf16)       # rstd row (per token)
            rs_cols = cst.tile([P, NTT], f32)  # rstd, token-partition layout
            dnall = cst.tile([16, 512], f32)   # softmax denominators per head
            rc_f = cst.tile([16, 512], f32)
            rc_bf = cst.tile([16, 512], bf16)
            b1_pp = cst.tile([P, HIDDEN // P], f32)
            nc.gpsimd.dma_start(b1_pp[:], b1pp_d[:])

            res1 = resp.tile([P, NTC, DIM], f32)   # attn residual stream
            h2t = resp.tile([P, NCT, 512], bf16)   # LN2(res1)^T for FC1
            b2_bc = resp.tile([P, DIM], f32)
            nc.gpsimd.dma_start(b2_bc[:], b2bc_d[:])

            # attention-lifetime SBUF (released after proj)
            atn = tc.alloc_tile_pool(name="atn", bufs=1)
            vsb = atn.tile([P, NTT, HEADS, 65], bf16)   # V-hat + ones col 64
            q_sb = atn.tile([P, NCT, 512], bf16)
            k_sb = atn.tile([P, NCT, T], bf16)
            osb = atn.tile([P, NCT, 512], bf16)         # per-pair attn out
            rb_sb = atn.tile([P, NTC, 512], bf16)       # rstd broadcast

            nc.vector.memset(
                vsb[:, :, :, 64:65].rearrange("p a b c -> p (a b c)"), 1.0)

            # raw x^T + K/Q weights (released before FC1 weight prefetch)
            xTp = tc.alloc_tile_pool(name="xTp", bufs=1, side="right")
            xT_sb = xTp.tile([P, NCT, T], bf16)
            kqw = tc.alloc_tile_pool(name="kqw", bufs=2, side="right")
            wkqc_sb = kqw.tile([2, NCT, 256], bf16, tag="wkqc", bufs=1)
            nc.gpsimd.dma_start(wkqc_sb[:], wkqc_d[:])

            kqp = tc.alloc_tile_pool(name="kqp", bufs=1, space="PSUM")
            wkq_tiles = {}

            def load_wkq(j):
                w = kqw.tile([P, NCT, 256], bf16, tag="wkq", name=f"wkq{j}")
                nc.sync.dma_start(w[:], wkq_d[j])
                wkq_tiles[j] = w

            def emit_k_chunk(j, tcn):
                w = wkq_tiles[j]
                kp = kqp.tile([P, 512], f32, tag="kq", name=f"kp{j}_{tcn}")
                sl = slice(tcn * 512, (tcn + 1) * 512)
                for k in range(NCT):
                    nc.tensor.matmul(kp[:], w[:, k, 0:P], xT_sb[:, k, sl],
                                     start=(k == 0), stop=False)
                nc.tensor.matmul(kp[:], wkqc_sb[:, j, 0:P], corr2[0:2, sl],
                                 start=False, stop=True)
                nc.vector.tensor_tensor(k_sb[:, j, sl], kp[:], rb_sb[:, tcn, :],
                                        ALU.mult)

            def emit_q(j, pop=True):
                w = wkq_tiles.pop(j) if pop else wkq_tiles[j]
                qp = kqp.tile([P, 512], f32, tag="kq", name=f"qp{j}")
                for k in range(NCT):
                    nc.tensor.matmul(qp[:], w[:, k, P:256], xT_sb[:, k, 0:512],
                                     start=(k == 0), stop=False)
                nc.tensor.matmul(qp[:], wkqc_sb[:, j, P:256], corr2[0:2, 0:512],
                                 start=False, stop=True)
                nc.vector.tensor_tensor(q_sb[:, j, :], qp[:], rb_sb[:, 0, :],
                                        ALU.mult)

            # ---------------- Phase A: LN1 stats + V-hat ----------------
            with tc.tile_pool(name="xsp", bufs=4) as xsp, \
                 tc.tile_pool(name="stp", bufs=2) as stp, \
                 tc.tile_pool(name="wvp", bufs=1) as wvp, \
                 tc.tile_pool(name="aps", bufs=1, space="PSUM") as aps, \
                 tc.tile_pool(name="rbp", bufs=1, space="PSUM") as rbp, \
                 tc.tile_pool(name="vps", bufs=4, space="PSUM") as vps:
                wv_sb = wvp.tile([P, NCT, DIM], bf16)
                wv1_sb = wvp.tile([1, DIM], bf16)

                def nc_wv1(oc):
                    return wv1_sb[0:1, oc * 512:(oc + 1) * 512]
                # stats tiles first (tiny, feed the corr rows), then wv (sync)
                # and x^T (scalar) slices dispatched on separate queues so the
                # first V-hat k-chain starts within ~2us
                xs_pre = {}
                for s in range(3):
                    xs = xsp.tile([P, DIM], bf16, tag="xs", name=f"xs{s}")
                    nc.sync.dma_start(xs[:], xsb_d[s * P:(s + 1) * P, :])
                    xs_pre[s] = xs
                # first two k-slices split column-wise so they spread over
                # several DMA queues (a whole 512KB slice on one queue takes
                # ~12us and gates the first V-hat matmul)
                for k in range(2):
                    for c in range(4):
                        nc.sync.dma_start(
                            wv_sb[:, k, c * 256:(c + 1) * 256],
                            wv_d[k * P:(k + 1) * P, c * 256:(c + 1) * 256])
                        nc.scalar.dma_start(
                            xT_sb[:, k, c * 512:(c + 1) * 512],
                            xTb_d[k * P:(k + 1) * P, c * 512:(c + 1) * 512])
                for k in range(2, NCT):
                    nc.sync.dma_start(wv_sb[:, k, :],
                                      wv_d[k * P:(k + 1) * P, :])
                    nc.scalar.dma_start(xT_sb[:, k, :],
                                        xTb_d[k * P:(k + 1) * P, :])
                nc.sync.dma_start(wv1_sb[:], wv1_d[:])
                nc.scalar.activation(warm[:], eps_t[:], AF.Sqrt)
                load_wkq(0)
                load_wkq(1)
                for s in range(NTT):
                    if s in xs_pre:
                        xs = xs_pre.pop(s)
                    else:
                        xs = xsp.tile([P, DIM], bf16, tag="xs", name=f"xs{s}")
                        nc.sync.dma_start(xs[:], xsb_d[s * P:(s + 1) * P, :])
                    stats = stp.tile([P, 2, 6], f32, tag="bst")
                    for g in range(2):
                        nc.vector.bn_stats(stats[:, g, :], xs[:, g * 512:(g + 1) * 512])
                    mv = stp.tile([P, 2], f32, tag="mv")
                    nc.vector.bn_aggr(mv[:], stats[:])
                    stk = stp.tile([P, 2], f32, tag="stk")
                    nc.vector.tensor_scalar(stk[:, 0:1], mv[:, 0:1], -1.0, None, ALU.mult)
                    nc.scalar.activation(stk[:, 1:2], mv[:, 1:2], AF.Sqrt, bias=eps_t[:])
                    nc.vector.reciprocal(rs_cols[:, s:s + 1], stk[:, 1:2])
                    # V-hat k-chains first: they only need x/wv tiles, so the
                    # PE is not gated on the stats chain above.
                    vp2 = []
                    for oc in range(2):
                        vp = vps.tile([P, 512], f32, tag="vp")
                        for k in range(NCT):
                            nc.tensor.matmul(vp[:], xT_sb[:, k, s * P:(s + 1) * P],
                                             wv_sb[:, k, oc * 512:(oc + 1) * 512],
                                             start=(k == 0), stop=False)
                        vp2.append(vp)
                    pst = aps.tile([2, P], f32, tag="pst")
                    nc.tensor.transpose(pst[:], stk[:], idf[:])
                    nc.vector.tensor_copy(corr2[:, s * P:(s + 1) * P], pst[:])
                    pst1 = aps.tile([1, P], f32, tag="pst1")
                    nc.tensor.transpose(pst1[:], rs_cols[:, s:s + 1], idf[:])
                    nc.vector.tensor_copy(rsr[:, s * P:(s + 1) * P], pst1[:])
                    for oc in range(2):
                        vp = vp2[oc]
                        nc.tensor.matmul(vp[:], corr2[0:1, s * P:(s + 1) * P],
                                         nc_wv1(oc),
                                         start=False, stop=True)
                        nc.vector.tensor_scalar(
                            vsb[:, s, oc * 8:(oc + 1) * 8, 0:64],
                            vp[:].rearrange("p (h d) -> p h d", d=64),
                            rs_cols[:, s:s + 1], None, ALU.mult)
                    if s % 4 == 3:
                        tcn = s // 4
                        rb = rbp.tile([P, 512], f32, tag="rb")
                        nc.tensor.matmul(rb[:], ones1b[:],
                                         rsr[0:1, tcn * 512:(tcn + 1) * 512],
                                         start=True, stop=True)
                        nc.vector.tensor_copy(rb_sb[:, tcn, :], rb[:])
                        # hide the first two K head-tiles + Q(0) under V-hat
                        emit_k_chunk(0, tcn)
                        emit_k_chunk(1, tcn)
                        if s == 7:
                            emit_q(0, pop=False)
                        if s == NTT - 1:
                            wkq_tiles.pop(0)
                            load_wkq(2)

            # ---------------- Phase B+C: scores/exp + PV + remaining K/Q ----------------
            psb_slots = {}
            w1_pool = []   # filled mid-phase once xT frees

            with tc.tile_pool(name="psbp", bufs=8) as psbp, \
                 tc.tile_pool(name="tvec", bufs=2) as tvec, \
                 tc.tile_pool(name="spp", bufs=2, space="PSUM") as spp, \
                 tc.tile_pool(name="pvp", bufs=3, space="PSUM") as pvp:

                def emit_s(j, kt):
                    sp = spp.tile([P, 2, 512], f32, tag="sp", name=f"sp{j}_{kt}")
                    ksl = slice(kt * P, (kt + 1) * P)
                    nc.tensor.matmul(sp[:, 0, :], k_sb[0:64, j, ksl],
                                     q_sb[0:64, j, :], start=True, stop=True)
                    nc.tensor.matmul(sp[:, 1, :], k_sb[64:128, j, ksl],
                                     q_sb[64:128, j, :], start=True, stop=True)
                    slot = psbp.tile([P, 2, 512], bf16, tag="psb",
                                     name=f"psb{j}_{kt}")
                    nc.scalar.activation(slot[:], sp[:], AF.Exp, scale=SCALE)
                    psb_slots[(j, kt)] = slot

                def emit_pv_kt(j, kt, pvA, pvB):
                    slot = psb_slots.pop((j, kt))
                    nc.tensor.matmul(pvA[:], vsb[:, kt, 2 * j, :], slot[:, 0, :],
                                     start=(kt == 0), stop=(kt == NTT - 1))
                    nc.tensor.matmul(pvB[:], vsb[:, kt, 2 * j + 1, :], slot[:, 1, :],
                                     start=(kt == 0), stop=(kt == NTT - 1))

                def emit_tail(h, pv):
                    dtmp = tvec.tile([65, 512], f32, tag="dtmp", name=f"dt{h}",
                                     bufs=2)
                    nc.scalar.activation(dtmp[64:65, :], pv[64:65, :], AF.Copy)
                    nc.gpsimd.dma_start(dnall[h:h + 1, :], dtmp[64:65, :])
                    if h % 2 == 0:
                        nc.vector.tensor_copy(osb[0:64, h // 2, :], pv[0:64, :])
                    else:
                        ot = tvec.tile([64, 512], bf16, tag="ot", name=f"ot{h}")
                        nc.vector.tensor_copy(ot[:], pv[0:64, :])
                        nc.gpsimd.dma_start(osb[64:128, h // 2, :], ot[:])

                LAG = 4
                for j in range(NCT):
                    if j == NCT - 1:
                        # x^T and K/Q weights are dead: free them; prefetch
                        # proj weights and the first w1 slice under the last
                        # scores/PV round.
                        kqw.release()
                        xTp.release()
                        w1a = tc.alloc_tile_pool(name="w1a", bufs=1,
                                                 side="right")
                        w1_sb_a = w1a.tile([P, 8, NCT, P], bf16)
                        for og in range(2):
                            nc.sync.dma_start(
                                w1_sb_a[:, og * 4:(og + 1) * 4, :, :],
                                w1h_d[og * 4:(og + 1) * 4].rearrange(
                                    "a p b c -> p a b c"))
                        w1_pool.append((w1a, w1_sb_a))
                        wpx = tc.alloc_tile_pool(name="wpx", bufs=1,
                                                 side="right")
                        wp_sb = wpx.tile([P, NCT, DIM], bf16)
                        nc.sync.dma_start(wp_sb[:], wp_d[:])
                        bp_bc = wpx.tile([P, DIM], f32)
                        nc.sync.dma_start(bp_bc[:], bpbc_d[:])
                    pvA = pvp.tile([65, 512], f32, tag="pv", name=f"pvA{j}")
                    pvB = pvp.tile([65, 512], f32, tag="pv", name=f"pvB{j}")
                    for kt in range(NTT):
                        emit_s(j, kt)
                        if kt == 1 and j + 2 < NCT:
                            emit_k_chunk(j + 2, 0)
                        elif kt == 5 and j + 2 < NCT:
                            emit_k_chunk(j + 2, 1)
                        elif kt == 9 and 2 <= j + 1 < NCT:
                            emit_k_chunk(j + 1, 2)
                        elif kt == 13 and 2 <= j + 1 < NCT:
                            emit_k_chunk(j + 1, 3)
                        elif kt == 14 and j + 1 < NCT:
                            emit_q(j + 1)
                            if j + 3 < NCT:
                                load_wkq(j + 3)
                        if kt >= LAG:
                            emit_pv_kt(j, kt - LAG, pvA, pvB)
                    for kt in range(NTT - LAG, NTT):
                        emit_pv_kt(j, kt, pvA, pvB)
                    emit_tail(2 * j, pvA)
                    emit_tail(2 * j + 1, pvB)

            # ---------------- Phase D: normalize o, proj + residual ----------------
            kqp.release()
            with tc.tile_pool(name="dvec", bufs=2) as dvec, \
                 tc.tile_pool(name="xrp", bufs=4) as xrp, \
                 tc.tile_pool(name="st2", bufs=2) as st2, \
                 tc.tile_pool(name="h2p", bufs=2) as h2p, \
                 tc.tile_pool(name="bcp", bufs=2, space="PSUM") as bcp, \
                 tc.tile_pool(name="tps", bufs=2, space="PSUM") as tps, \
                 tc.tile_pool(name="pjp", bufs=2, space="PSUM") as pjp:
                xres_t = []
                for ts in range(NTC):
                    xres = xrp.tile([P, DIM], f32, tag="xres", name=f"xr{ts}")
                    nc.sync.dma_start(xres[:], xtm_d[ts * P:(ts + 1) * P, :])
                    xres_t.append(xres)
                nc.vector.reciprocal_approx_fast(rc_f[:], dnall[:])
                nc.vector.tensor_copy(rc_bf[:], rc_f[:])
                for j in range(NCT):
                    bc = bcp.tile([P, 512], f32, tag="bc", name=f"bc{j}")
                    nc.tensor.matmul(bc[:], sel[:, j, :], rc_bf[:],
                                     start=True, stop=True)
                    rcb = dvec.tile([P, 512], bf16, tag="rcb", name=f"rcb{j}")
                    # alternate engines so the 8 psum->sbuf copies run 2-wide
                    if j % 2 == 0:
                        nc.scalar.activation(rcb[:], bc[:], AF.Copy)
                    else:
                        nc.vector.tensor_copy(rcb[:], bc[:])
                    nc.vector.tensor_tensor(osb[:, j, :], osb[:, j, :], rcb[:],
                                            ALU.mult)
                for ts in range(NTC):
                    xres = xres_t[ts]
                    for oc in range(2):
                        pj = pjp.tile([P, 512], f32, tag="pj")
                        for j in range(NCT):
                            nc.tensor.matmul(pj[:], osb[:, j, ts * P:(ts + 1) * P],
                                             wp_sb[:, j, oc * 512:(oc + 1) * 512],
                                             start=(j == 0), stop=(j == NCT - 1))
                        if zb_proj:
                            nc.vector.tensor_tensor(
                                res1[:, ts, oc * 512:(oc + 1) * 512], pj[:],
                                xres[:, oc * 512:(oc + 1) * 512], ALU.add)
                        else:
                            t1 = dvec.tile([P, 512], f32, tag="t1")
                            nc.vector.tensor_tensor(
                                t1[:], pj[:],
                                xres[:, oc * 512:(oc + 1) * 512], ALU.add)
                            nc.vector.tensor_tensor(
                                res1[:, ts, oc * 512:(oc + 1) * 512], t1[:],
                                bp_bc[:, oc * 512:(oc + 1) * 512], ALU.add)
                    # LN2 + transpose for this token tile, hidden under the
                    # proj matmuls of the next one
                    stats2 = st2.tile([P, 2, 6], f32, tag="bst2")
                    for g in range(2):
                        nc.vector.bn_stats(stats2[:, g, :],
                                           res1[:, ts, g * 512:(g + 1) * 512])
                    mv2 = st2.tile([P, 2], f32, tag="mv2")
                    nc.vector.bn_aggr(mv2[:], stats2[:])
                    sdv2 = st2.tile([P, 1], f32, tag="sdv2")
                    nc.scalar.activation(sdv2[:], mv2[:, 1:2], AF.Sqrt, bias=eps_t[:])
                    rs2 = st2.tile([P, 1], f32, tag="rs2")
                    nc.vector.reciprocal(rs2[:], sdv2[:])
                    nmu = st2.tile([P, 1], f32, tag="nmu")
                    nc.vector.tensor_tensor(nmu[:], mv2[:, 0:1], rs2[:], ALU.mult)
                    nc.vector.tensor_scalar(nmu[:], nmu[:], -1.0, None, ALU.mult)
                    h2 = h2p.tile([P, DIM], bf16, tag="h2")
                    nc.scalar.activation(h2[:], res1[:, ts, :], AF.Identity,
                                         bias=nmu[:], scale=rs2[:])
                    for ct in range(NCT):
                        tp = tps.tile([P, P], bf16, tag="tp")
                        nc.tensor.transpose(tp[:], h2[:, ct * P:(ct + 1) * P], idb[:])
                        if ct % 2 == 0:
                            nc.vector.tensor_copy(
                                h2t[:, ct, ts * P:(ts + 1) * P], tp[:])
                        else:
                            nc.scalar.activation(
                                h2t[:, ct, ts * P:(ts + 1) * P], tp[:], AF.Copy)

            atn.release()
            wpx.release()

            # rest of w1 (space freed by attention buffers)
            w1b = tc.alloc_tile_pool(name="w1b", bufs=1)
            w1_sb_b = w1b.tile([P, 24, NCT, P], bf16)
            for og in range(6):
                nc.sync.dma_start(
                    w1_sb_b[:, og * 4:(og + 1) * 4, :, :],
                    w1h_d[8 + og * 4:8 + (og + 1) * 4].rearrange(
                        "a p b c -> p a b c"))

            # ---------------- Phase E: LN2 + MLP ----------------
            with tc.tile_pool(name="h3p", bufs=1) as h3p, \
                 tc.tile_pool(name="w2p", bufs=6) as w2p, \
                 tc.tile_pool(name="mlv", bufs=2) as mlv, \
                 tc.tile_pool(name="f1s", bufs=2, space="PSUM") as f1s, \
                 tc.tile_pool(name="f2s", bufs=4, space="PSUM") as f2s:
                h3t = h3p.tile([P, HIDDEN // P, 512], bf16)
                if zb_fc2:
                    resb = res1
                else:
                    resb = h3p.tile([P, NTC, DIM], f32)
                    for ts in range(NTC):
                        nc.gpsimd.tensor_tensor(resb[:, ts, :], res1[:, ts, :],
                                                b2_bc[:], ALU.add)

                # fc1 + gelu -> h3t (hidden-major)
                for ot in range(HIDDEN // P):
                    w1_sb = w1_pool[0][1] if ot < 8 else w1_sb_b
                    oti = ot if ot < 8 else ot - 8
                    fp = f1s.tile([P, 512], f32, tag="f1")
                    for ct in range(NCT):
                        nc.tensor.matmul(fp[:], w1_sb[:, oti, ct, :], h2t[:, ct, :],
                                         start=(ct == 0), stop=(ct == NCT - 1))
                    nc.scalar.activation(h3t[:, ot, :], fp[:], AF.Gelu,
                                         bias=b1_pp[:, ot:ot + 1])

                # fc2 + bias + residual -> out
                for oc in range(2):
                    f2t = [f2s.tile([P, 512], f32, tag="f2", name=f"f2_{oc}_{i}")
                           for i in range(NTC)]
                    for cg in range(HIDDEN // P // 2):
                        w2t = w2p.tile([P, 2, 512], bf16, tag="w2t")
                        nc.scalar.dma_start(
                            w2t[:], w2T_d[cg * 256:(cg + 1) * 256,
                                          oc * 512:(oc + 1) * 512]
                            .rearrange("(a p) o -> p a o", p=P))
                        for ci in range(2):
                            ct = cg * 2 + ci
                            for ts in range(NTC):
                                nc.tensor.matmul(f2t[ts][:],
                                                 h3t[:, ct, ts * P:(ts + 1) * P],
                                                 w2t[:, ci, :], start=(ct == 0),
                                                 stop=(ct == HIDDEN // P - 1))
                    for ts in range(NTC):
                        t2 = mlv.tile([P, 512], f32, tag="t22")
                        nc.vector.tensor_tensor(
                            t2[:], f2t[ts][:],
                            resb[:, ts, oc * 512:(oc + 1) * 512], ALU.add)
                        nc.gpsimd.dma_start(
                            out_d[ts * P:(ts + 1) * P, oc * 512:(oc + 1) * 512],
                            t2[:])

            w1_pool[0][0].release()
            w1b.release()

    nc.compile()
    return nc


def _get_program(zb_proj, zb_fc2):
    global _PROG
    key = (zb_proj, zb_fc2)
    if _PROG is None or _PROG[0] != key:
        _PROG = (key, _build_program(zb_proj, zb_fc2))
    return _PROG[1]


def _pack_cols(wT):
    """[C, O] -> [O//128, 128(p), C//128(k), 128(o)] so each o-tile DMA is contiguous."""
    C, O = wT.shape
    # out[ot, p, k, o] = wT[k*128+p, ot*128+o]
    return np.ascontiguousarray(
        wT.reshape(C // P, P, O // P, P).transpose(2, 1, 0, 3))


def _host_prep(x, ln1_g, ln1_b, w_qkv, w_proj, b_proj, ln2_g, ln2_b,
               w_fc1, b_fc1, w_fc2, b_fc2):
    """Per-core input dicts. Pure layout/weight-folding work (no activation math)."""
    f = np.float32
    bf = ml_dtypes.bfloat16
    x = np.asarray(x, f)
    g1 = np.asarray(ln1_g, f); b1 = np.asarray(ln1_b, f)
    g2 = np.asarray(ln2_g, f); b2 = np.asarray(ln2_b, f)
    w_qkv = np.asarray(w_qkv, f); w_proj = np.asarray(w_proj, f)
    w_fc1 = np.asarray(w_fc1, f); w_fc2 = np.asarray(w_fc2, f)
    b_proj = np.asarray(b_proj, f); b_fc1 = np.asarray(b_fc1, f)
    b_fc2 = np.asarray(b_fc2, f)

    wq, wk, wv = w_qkv[0:DIM], w_qkv[DIM:2 * DIM], w_qkv[2 * DIM:3 * DIM]
    wqg = (wq * g1[None, :]).T   # [C(f), O] feature-major, LN gain folded
    wkg = (wk * g1[None, :]).T
    wvg = (wv * g1[None, :]).T
    bq = wq @ b1; bk = wk @ b1; bv = wv @ b1

    wkq = np.concatenate([_pack_cols(wkg), _pack_cols(wqg)], axis=3)
    wkqc = np.stack([
        np.concatenate([wkg.sum(axis=0).reshape(NCT, P),
                        wqg.sum(axis=0).reshape(NCT, P)], axis=1),
        np.concatenate([bk.reshape(NCT, P), bq.reshape(NCT, P)], axis=1),
    ], axis=0)  # [2, 8, 256]

    sel = np.zeros((16, NCT, P), f)
    for j in range(NCT):
        sel[2 * j, j, 0:64] = 1.0
        sel[2 * j + 1, j, 64:128] = 1.0

    bp_total = b_proj + w_proj @ bv
    shared = {
        "wv_d": np.ascontiguousarray(wvg.astype(bf)),
        "wv1_d": np.ascontiguousarray(wvg.sum(axis=0).reshape(1, DIM).astype(bf)),
        "wkq_d": np.ascontiguousarray(wkq.astype(bf)),
        "wkqc_d": np.ascontiguousarray(wkqc.astype(bf)),
        "selb_d": np.ascontiguousarray(sel.astype(bf)),
        "wp_d": np.ascontiguousarray(
            w_proj.T.reshape(NCT, P, DIM).transpose(1, 0, 2).astype(bf)),
        "bpbc_d": np.ascontiguousarray(np.broadcast_to(bp_total, (P, DIM)).astype(f)),
        "w1h_d": np.ascontiguousarray(_pack_cols((w_fc1 * g2[None, :]).T).astype(bf)),
        "b1pp_d": np.ascontiguousarray(
            (b_fc1 + w_fc1 @ b2).reshape(HIDDEN // P, P).T.astype(f)),
        "w2T_d": np.ascontiguousarray(w_fc2.T.astype(bf)),
        "b2bc_d": np.ascontiguousarray(np.broadcast_to(b_fc2, (P, DIM)).astype(f)),
    }
    in_maps = []
    for core in range(8):
        b, q = core // 4, core % 4
        xroll = np.roll(x[b], -CH * q, axis=0)
        m = dict(shared)
        m["xtm_d"] = np.ascontiguousarray(xroll)
        xb = xroll.astype(bf)
        m["xsb_d"] = np.ascontiguousarray(xb)
        m["xTb_d"] = np.ascontiguousarray(xb.T)
        in_maps.append(m)
    return in_maps


def kernel(**inputs) -> np.ndarray:
    _setup_env()
    from concourse import bass_utils

    in_maps = _host_prep(**inputs)
    zb_proj = not np.any(in_maps[0]["bpbc_d"])
    zb_fc2 = not np.any(in_maps[0]["b2bc_d"])
    nc = _get_program(zb_proj, zb_fc2)
    run_kwargs = {}
    if os.environ.get("BASS_PROFILE"):
        import tempfile
        run_kwargs = dict(trace=True, tmpdir=tempfile.mkdtemp(prefix="blk_prof"))
    res = bass_utils.run_bass_kernel_spmd(nc, in_maps, core_ids=list(range(8)),
                                          **run_kwargs)
    kernel.last_result = res
    out = np.empty((2, T, DIM), np.float32)
    for core in range(8):
        b, q = core // 4, core % 4
        out[b, CH * q:CH * (q + 1), :] = res.results[core]["out_d"]
    return out


# revision 23
# speedup vs baseline: 1.0022x; 1.0022x over previous
"""Trainium2 Bass kernel for a pre-norm transformer block (nn_Block_74766790689102).

v2 strategy (8 NeuronCores, zero-communication SPMD):
  core c handles batch b=c//4, query chunk q=c%4 (512 of 2048 tokens); inputs
  are host-rotated so each core's chunk sits at token positions 0:512 -> one
  identical SPMD program for all 8 cores. Each core redundantly computes K/V
  for its whole batch (attention needs all keys).

  Changes vs v1 (716us):
  - bf16 operands for every matmul (fp32 PSUM accumulation): halves weight
    DMA and enables fast weight loads on the PE.
  - Normalized x is never materialized. QKV matmuls consume raw x^T plus a
    rank-2 correction row ((-mu, sdv) x (col-sums, bias)) folded into the
    contraction, then a per-token rstd scale on the outputs. Kills the
    gpsimd/vector normalize traffic and one full 8MB x reload.
  - K, exp'd scores, attention outputs, res1 all stay in SBUF (no DRAM
    roundtrips).
  - K production for head-pair j+1, scores/exp for j, and PV for j-1 are
    interleaved in one PE stream so the ~134us of scalar-engine exp hides
    behind PE work.
  - Softmax denominators collect into one [16,512] tile; a single
    reciprocal_approx_fast + 8 PE broadcasts replace 16 serial [1,512] DVE
    reciprocals (was 52us).
  - V/proj biases fold into the proj bias on the host (softmax rows sum to 1).
  - w1 is fully resident in SBUF before FC1 starts; w2 streams as bf16.
"""

import os
import sys
import types

import numpy as np
import ml_dtypes

DIM = 1024
HEADS = 16
HD = 64
HIDDEN = 4096
T = 2048          # tokens per batch
CH = 512          # chunk tokens per core
SCALE = HD ** -0.5
EPS = 1e-5
NCT = DIM // 128  # 8 feature tiles
NTC = T // 512    # 4 token chunks
NTT = T // 128    # 16 token tiles
P = 128

_ENV_READY = False
_PROG = None


def _setup_env():
    global _ENV_READY
    if _ENV_READY:
        return
    if "/opt/trn_rl_repo" not in sys.path:
        sys.path.insert(0, "/opt/trn_rl_repo")
    # NTFF profile hook shim (the RL container's antenv lacks axon_hooks).
    try:
        import antenv
        if "antenv.axon_hooks" not in sys.modules:
            mod = types.ModuleType("antenv.axon_hooks")
            mod._hook = None
            mod.set_axon_ntff_profile_hook = lambda h: setattr(mod, "_hook", h)
            mod.get_axon_ntff_profile_hook = lambda: mod._hook
            sys.modules["antenv.axon_hooks"] = mod
            antenv.axon_hooks = mod
        if os.environ.get("BASS_PROFILE"):
            from trn_agent_boot.trn_boot import _ntff_profile_via_ctypes
            sys.modules["antenv.axon_hooks"].set_axon_ntff_profile_hook(
                _ntff_profile_via_ctypes("/opt/axon/libaxon_pjrt.so"))
    except Exception:
        pass
    _ENV_READY = True


def _build_program(zb_proj=False, zb_fc2=False):
    """Build + compile the single-core Bass program (same for all 8 cores).

    zb_proj/zb_fc2: host-verified all-zero proj/fc2 biases let the epilogues
    skip the dead bias adds (general path kept as fallback).
    """
    _setup_env()
    import concourse.bacc as bacc
    import concourse.tile as tile
    import concourse.mybir as mybir
    from concourse.masks import make_identity

    dt = mybir.dt
    AF = mybir.ActivationFunctionType
    ALU = mybir.AluOpType
    f32, bf16 = dt.float32, dt.bfloat16

    nc = bacc.Bacc("TRN2", target_bir_lowering=False, debug=False, num_devices=8)

    # ---- I/O ----
    xtm_d = nc.dram_tensor("xtm_d", [T, DIM], f32, kind="ExternalInput").ap()
    xsb_d = nc.dram_tensor("xsb_d", [T, DIM], bf16, kind="ExternalInput").ap()
    xTb_d = nc.dram_tensor("xTb_d", [DIM, T], bf16, kind="ExternalInput").ap()
    wv_d = nc.dram_tensor("wv_d", [DIM, DIM], bf16, kind="ExternalInput").ap()
    wv1_d = nc.dram_tensor("wv1_d", [1, DIM], bf16, kind="ExternalInput").ap()
    wkq_d = nc.dram_tensor("wkq_d", [NCT, P, NCT, 256], bf16, kind="ExternalInput").ap()
    wkqc_d = nc.dram_tensor("wkqc_d", [2, NCT, 256], bf16, kind="ExternalInput").ap()
    selb_d = nc.dram_tensor("selb_d", [16, NCT, P], bf16, kind="ExternalInput").ap()
    wp_d = nc.dram_tensor("wp_d", [P, NCT, DIM], bf16, kind="ExternalInput").ap()
    bpbc_d = nc.dram_tensor("bpbc_d", [P, DIM], f32, kind="ExternalInput").ap()
    w1h_d = nc.dram_tensor("w1h_d", [HIDDEN // P, P, NCT, P], bf16, kind="ExternalInput").ap()
    b1pp_d = nc.dram_tensor("b1pp_d", [P, HIDDEN // P], f32, kind="ExternalInput").ap()
    w2T_d = nc.dram_tensor("w2T_d", [HIDDEN, DIM], bf16, kind="ExternalInput").ap()
    b2bc_d = nc.dram_tensor("b2bc_d", [P, DIM], f32, kind="ExternalInput").ap()
    out_d = nc.dram_tensor("out_d", [CH, DIM], f32, kind="ExternalOutput").ap()

    with tile.TileContext(nc) as tc:
        with tc.tile_pool(name="cst", bufs=1) as cst, \
             tc.tile_pool(name="resp", bufs=1) as resp:

            # ---------------- constants ----------------
            idf = cst.tile([P, P], f32)
            make_identity(nc, idf[:])
            idb = cst.tile([P, P], bf16)
            nc.vector.tensor_copy(idb[:], idf[:])
            ones1b = cst.tile([1, P], bf16)
            nc.vector.memset(ones1b[:], 1.0)
            eps_t = cst.tile([P, 1], f32)
            nc.vector.memset(eps_t[:], EPS)
            warm = cst.tile([P, 1], f32)
            # head-pair selection matrices for the denominator broadcast,
            # split lo/hi so each half is a base-partition-0 matmul operand
            sel_lo = cst.tile([8, NCT, P], bf16)
            nc.gpsimd.dma_start(sel_lo[:], selb_d[0:8])
            sel_hi = cst.tile([8, NCT, P], bf16)
            nc.gpsimd.dma_start(sel_hi[:], selb_d[8:16])
            corr2 = cst.tile([2, T], bf16)     # rows: -mu, sdv (per token)
            rsr = cst.tile([1, T], bf16)       # rstd row (per token)
            rs_cols = cst.tile([P, NTT], f32)  # rstd, token-partition layout
            dn_lo = cst.tile([8, 512], f32)    # denominators, heads 0-7
            dn_hi = cst.tile([8, 512], f32)    # denominators, heads 8-15
            rc_lo_f = cst.tile([8, 512], f32)
            rc_hi_f = cst.tile([8, 512], f32)
            rc_lo_b = cst.tile([8, 512], bf16)
            rc_hi_b = cst.tile([8, 512], bf16)
            b1_pp = cst.tile([P, HIDDEN // P], f32)
            nc.gpsimd.dma_start(b1_pp[:], b1pp_d[:])

            res1 = resp.tile([P, NTC, DIM], f32)   # attn residual stream
            h2t = resp.tile([P, NCT, 512], bf16)   # LN2(res1)^T for FC1
            b2_bc = resp.tile([P, DIM], f32)
            nc.gpsimd.dma_start(b2_bc[:], b2bc_d[:])

            # attention-lifetime SBUF (released after proj)
            atn = tc.alloc_tile_pool(name="atn", bufs=1)
            vsb = atn.tile([P, NTT, HEADS, 65], bf16)   # V-hat + ones col 64
            q_sb = atn.tile([P, NCT, 512], bf16)
            k_sb = atn.tile([P, NCT, T], bf16)
            osb = atn.tile([P, NCT, 512], bf16)         # per-pair attn out
            rb_sb = atn.tile([P, NTC, 512], bf16)       # rstd broadcast

            nc.vector.memset(
                vsb[:, :, :, 64:65].rearrange("p a b c -> p (a b c)"), 1.0)

            # raw x^T + K/Q weights (released before FC1 weight prefetch)
            xTp = tc.alloc_tile_pool(name="xTp", bufs=1, side="right")
            xT_sb = xTp.tile([P, NCT, T], bf16)
            kqw = tc.alloc_tile_pool(name="kqw", bufs=2, side="right")
            wkqc_sb = kqw.tile([2, NCT, 256], bf16, tag="wkqc", bufs=1)
            nc.gpsimd.dma_start(wkqc_sb[:], wkqc_d[:])

            kqp = tc.alloc_tile_pool(name="kqp", bufs=1, space="PSUM")
            wkq_tiles = {}

            def load_wkq(j):
                w = kqw.tile([P, NCT, 256], bf16, tag="wkq", name=f"wkq{j}")
                nc.sync.dma_start(w[:], wkq_d[j])
                wkq_tiles[j] = w

            def emit_k_chunk(j, tcn):
                w = wkq_tiles[j]
                kp = kqp.tile([P, 512], f32, tag="kq", name=f"kp{j}_{tcn}")
                sl = slice(tcn * 512, (tcn + 1) * 512)
                for k in range(NCT):
                    nc.tensor.matmul(kp[:], w[:, k, 0:P], xT_sb[:, k, sl],
                                     start=(k == 0), stop=False)
                nc.tensor.matmul(kp[:], wkqc_sb[:, j, 0:P], corr2[0:2, sl],
                                 start=False, stop=True)
                nc.vector.tensor_tensor(k_sb[:, j, sl], kp[:], rb_sb[:, tcn, :],
                                        ALU.mult)

            def emit_q(j, pop=True):
                w = wkq_tiles.pop(j) if pop else wkq_tiles[j]
                qp = kqp.tile([P, 512], f32, tag="kq", name=f"qp{j}")
                for k in range(NCT):
                    nc.tensor.matmul(qp[:], w[:, k, P:256], xT_sb[:, k, 0:512],
                                     start=(k == 0), stop=False)
                nc.tensor.matmul(qp[:], wkqc_sb[:, j, P:256], corr2[0:2, 0:512],
                                 start=False, stop=True)
                nc.vector.tensor_tensor(q_sb[:, j, :], qp[:], rb_sb[:, 0, :],
                                        ALU.mult)

            # ---------------- Phase A: LN1 stats + V-hat ----------------
            with tc.tile_pool(name="xsp", bufs=4) as xsp, \
                 tc.tile_pool(name="stp", bufs=2) as stp, \
                 tc.tile_pool(name="wvp", bufs=1) as wvp, \
                 tc.tile_pool(name="aps", bufs=1, space="PSUM") as aps, \
                 tc.tile_pool(name="rbp", bufs=1, space="PSUM") as rbp, \
                 tc.tile_pool(name="vps", bufs=4, space="PSUM") as vps:
                wv_sb = wvp.tile([P, NCT, DIM], bf16)
                wv1_sb = wvp.tile([1, DIM], bf16)

                def nc_wv1(oc):
                    return wv1_sb[0:1, oc * 512:(oc + 1) * 512]
                # stats tiles first (tiny, feed the corr rows), then wv (sync)
                # and x^T (scalar) slices dispatched on separate queues so the
                # first V-hat k-chain starts within ~2us
                xs_pre = {}
                for s in range(3):
                    xs = xsp.tile([P, DIM], bf16, tag="xs", name=f"xs{s}")
                    nc.sync.dma_start(xs[:], xsb_d[s * P:(s + 1) * P, :])
                    xs_pre[s] = xs
                # first two k-slices split column-wise so they spread over
                # several DMA queues (a whole 512KB slice on one queue takes
                # ~12us and gates the first V-hat matmul)
                for k in range(2):
                    for c in range(4):
                        nc.sync.dma_start(
                            wv_sb[:, k, c * 256:(c + 1) * 256],
                            wv_d[k * P:(k + 1) * P, c * 256:(c + 1) * 256])
                        nc.scalar.dma_start(
                            xT_sb[:, k, c * 512:(c + 1) * 512],
                            xTb_d[k * P:(k + 1) * P, c * 512:(c + 1) * 512])
                for k in range(2, 4):
                    for c in range(2):
                        nc.sync.dma_start(
                            wv_sb[:, k, c * 512:(c + 1) * 512],
                            wv_d[k * P:(k + 1) * P, c * 512:(c + 1) * 512])
                        nc.scalar.dma_start(
                            xT_sb[:, k, c * 1024:(c + 1) * 1024],
                            xTb_d[k * P:(k + 1) * P, c * 1024:(c + 1) * 1024])
                for k in range(4, NCT):
                    nc.sync.dma_start(wv_sb[:, k, :],
                                      wv_d[k * P:(k + 1) * P, :])
                    nc.scalar.dma_start(xT_sb[:, k, :],
                                        xTb_d[k * P:(k + 1) * P, :])
                nc.sync.dma_start(wv1_sb[:], wv1_d[:])
                nc.scalar.activation(warm[:], eps_t[:], AF.Sqrt)
                load_wkq(0)
                load_wkq(1)
                for s in range(NTT):
                    if s in xs_pre:
                        xs = xs_pre.pop(s)
                    else:
                        xs = xsp.tile([P, DIM], bf16, tag="xs", name=f"xs{s}")
                        nc.sync.dma_start(xs[:], xsb_d[s * P:(s + 1) * P, :])
                    stats = stp.tile([P, 2, 6], f32, tag="bst")
                    for g in range(2):
                        nc.vector.bn_stats(stats[:, g, :], xs[:, g * 512:(g + 1) * 512])
                    mv = stp.tile([P, 2], f32, tag="mv")
                    nc.vector.bn_aggr(mv[:], stats[:])
                    stk = stp.tile([P, 2], f32, tag="stk")
                    nc.vector.tensor_scalar(stk[:, 0:1], mv[:, 0:1], -1.0, None, ALU.mult)
                    nc.scalar.activation(stk[:, 1:2], mv[:, 1:2], AF.Sqrt, bias=eps_t[:])
                    nc.vector.reciprocal(rs_cols[:, s:s + 1], stk[:, 1:2])
                    # V-hat k-chains first: they only need x/wv tiles, so the
                    # PE is not gated on the stats chain above.
                    vp2 = []
                    for oc in range(2):
                        vp = vps.tile([P, 512], f32, tag="vp")
                        for k in range(NCT):
                            nc.tensor.matmul(vp[:], xT_sb[:, k, s * P:(s + 1) * P],
                                             wv_sb[:, k, oc * 512:(oc + 1) * 512],
                                             start=(k == 0), stop=False)
                        vp2.append(vp)
                    pst = aps.tile([2, P], f32, tag="pst")
                    nc.tensor.transpose(pst[:], stk[:], idf[:])
                    nc.vector.tensor_copy(corr2[:, s * P:(s + 1) * P], pst[:])
                    pst1 = aps.tile([1, P], f32, tag="pst1")
                    nc.tensor.transpose(pst1[:], rs_cols[:, s:s + 1], idf[:])
                    nc.vector.tensor_copy(rsr[:, s * P:(s + 1) * P], pst1[:])
                    for oc in range(2):
                        vp = vp2[oc]
                        nc.tensor.matmul(vp[:], corr2[0:1, s * P:(s + 1) * P],
                                         nc_wv1(oc),
                                         start=False, stop=True)
                        nc.vector.tensor_scalar(
                            vsb[:, s, oc * 8:(oc + 1) * 8, 0:64],
                            vp[:].rearrange("p (h d) -> p h d", d=64),
                            rs_cols[:, s:s + 1], None, ALU.mult)
                    if s % 4 == 3:
                        tcn = s // 4
                        rb = rbp.tile([P, 512], f32, tag="rb")
                        nc.tensor.matmul(rb[:], ones1b[:],
                                         rsr[0:1, tcn * 512:(tcn + 1) * 512],
                                         start=True, stop=True)
                        nc.vector.tensor_copy(rb_sb[:, tcn, :], rb[:])
                        # hide the first two K head-tiles + Q(0) under V-hat
                        emit_k_chunk(0, tcn)
                        emit_k_chunk(1, tcn)
                        if s == 7:
                            emit_q(0, pop=False)
                        if s == NTT - 1:
                            wkq_tiles.pop(0)
                            load_wkq(2)

            # ---------------- Phase B+C: scores/exp + PV + remaining K/Q ----------------
            psb_slots = {}
            w1_pool = []   # filled mid-phase once xT frees

            with tc.tile_pool(name="psbp", bufs=8) as psbp, \
                 tc.tile_pool(name="tvec", bufs=2) as tvec, \
                 tc.tile_pool(name="spp", bufs=2, space="PSUM") as spp, \
                 tc.tile_pool(name="pvp", bufs=3, space="PSUM") as pvp:

                def emit_s(j, kt):
                    sp = spp.tile([P, 2, 512], f32, tag="sp", name=f"sp{j}_{kt}")
                    ksl = slice(kt * P, (kt + 1) * P)
                    nc.tensor.matmul(sp[:, 0, :], k_sb[0:64, j, ksl],
                                     q_sb[0:64, j, :], start=True, stop=True)
                    nc.tensor.matmul(sp[:, 1, :], k_sb[64:128, j, ksl],
                                     q_sb[64:128, j, :], start=True, stop=True)
                    slot = psbp.tile([P, 2, 512], bf16, tag="psb",
                                     name=f"psb{j}_{kt}")
                    nc.scalar.activation(slot[:], sp[:], AF.Exp, scale=SCALE)
                    psb_slots[(j, kt)] = slot

                def emit_pv_kt(j, kt, pvA, pvB):
                    slot = psb_slots.pop((j, kt))
                    nc.tensor.matmul(pvA[:], vsb[:, kt, 2 * j, :], slot[:, 0, :],
                                     start=(kt == 0), stop=(kt == NTT - 1))
                    nc.tensor.matmul(pvB[:], vsb[:, kt, 2 * j + 1, :], slot[:, 1, :],
                                     start=(kt == 0), stop=(kt == NTT - 1))

                def emit_tail(h, pv):
                    dtmp = tvec.tile([65, 512], f32, tag="dtmp", name=f"dt{h}",
                                     bufs=2)
                    nc.scalar.activation(dtmp[64:65, :], pv[64:65, :], AF.Copy)
                    dnt = dn_lo if h < 8 else dn_hi
                    nc.gpsimd.dma_start(dnt[h % 8:h % 8 + 1, :], dtmp[64:65, :])
                    if h % 2 == 0:
                        nc.vector.tensor_copy(osb[0:64, h // 2, :], pv[0:64, :])
                    else:
                        ot = tvec.tile([64, 512], bf16, tag="ot", name=f"ot{h}")
                        nc.vector.tensor_copy(ot[:], pv[0:64, :])
                        nc.gpsimd.dma_start(osb[64:128, h // 2, :], ot[:])

                LAG = 4

                def emit_osb_scale(j, sel_t, rc_b, pool, tag):
                    bc = pool.tile([P, 512], f32, tag=tag, name=f"bcs{j}")
                    nc.tensor.matmul(bc[:], sel_t[:, j, :], rc_b[:],
                                     start=True, stop=True)
                    rcb = tvec.tile([P, 512], bf16, tag="rcb", name=f"rcb{j}",
                                    bufs=2)
                    nc.vector.tensor_copy(rcb[:], bc[:])
                    nc.vector.tensor_tensor(osb[:, j, :], osb[:, j, :], rcb[:],
                                            ALU.mult)

                for j in range(NCT):
                    if j == NCT - 1:
                        # x^T and K/Q weights are dead: free them; prefetch
                        # proj weights and the first w1 slice under the last
                        # scores/PV round.
                        kqw.release()
                        xTp.release()
                        w1a = tc.alloc_tile_pool(name="w1a", bufs=1,
                                                 side="right")
                        w1_sb_a = w1a.tile([P, 8, NCT, P], bf16)
                        for og in range(2):
                            nc.sync.dma_start(
                                w1_sb_a[:, og * 4:(og + 1) * 4, :, :],
                                w1h_d[og * 4:(og + 1) * 4].rearrange(
                                    "a p b c -> p a b c"))
                        w1_pool.append((w1a, w1_sb_a))
                        wpx = tc.alloc_tile_pool(name="wpx", bufs=1,
                                                 side="right")
                        wp_sb = wpx.tile([P, NCT, DIM], bf16)
                        nc.sync.dma_start(wp_sb[:], wp_d[:])
                        bp_bc = wpx.tile([P, DIM], f32)
                        nc.sync.dma_start(bp_bc[:], bpbc_d[:])
                    if j == NCT - 1:
                        # pairs 0-3 finished long ago: normalize their attn
                        # outputs now, hidden under the last scores round
                        nc.vector.reciprocal_approx_fast(rc_lo_f[:], dn_lo[:])
                        nc.vector.tensor_copy(rc_lo_b[:], rc_lo_f[:])
                        for jj in range(4):
                            emit_osb_scale(jj, sel_lo, rc_lo_b, kqp, "kq")
                    pvA = pvp.tile([65, 512], f32, tag="pv", name=f"pvA{j}")
                    pvB = pvp.tile([65, 512], f32, tag="pv", name=f"pvB{j}")
                    for kt in range(NTT):
                        emit_s(j, kt)
                        if kt == 1 and j + 2 < NCT:
                            emit_k_chunk(j + 2, 0)
                        elif kt == 5 and j + 2 < NCT:
                            emit_k_chunk(j + 2, 1)
                        elif kt == 9 and 2 <= j + 1 < NCT:
                            emit_k_chunk(j + 1, 2)
                        elif kt == 13 and 2 <= j + 1 < NCT:
                            emit_k_chunk(j + 1, 3)
                        elif kt == 14 and j + 1 < NCT:
                            emit_q(j + 1)
                            if j + 3 < NCT:
                                load_wkq(j + 3)
                        if kt >= LAG:
                            emit_pv_kt(j, kt - LAG, pvA, pvB)
                    for kt in range(NTT - LAG, NTT):
                        emit_pv_kt(j, kt, pvA, pvB)
                    emit_tail(2 * j, pvA)
                    emit_tail(2 * j + 1, pvB)

            # ---------------- Phase D: normalize o, proj + residual ----------------
            kqp.release()
            with tc.tile_pool(name="dvec", bufs=2) as dvec, \
                 tc.tile_pool(name="xrp", bufs=4) as xrp, \
                 tc.tile_pool(name="st2", bufs=2) as st2, \
                 tc.tile_pool(name="h2p", bufs=2) as h2p, \
                 tc.tile_pool(name="bcp", bufs=2, space="PSUM") as bcp, \
                 tc.tile_pool(name="tps", bufs=2, space="PSUM") as tps, \
                 tc.tile_pool(name="pjp", bufs=2, space="PSUM") as pjp:
                xres_t = []
                for ts in range(NTC):
                    xres = xrp.tile([P, DIM], f32, tag="xres", name=f"xr{ts}")
                    nc.sync.dma_start(xres[:], xtm_d[ts * P:(ts + 1) * P, :])
                    xres_t.append(xres)
                nc.vector.reciprocal_approx_fast(rc_hi_f[:], dn_hi[:])
                nc.vector.tensor_copy(rc_hi_b[:], rc_hi_f[:])
                for j in range(4, NCT):
                    bc = bcp.tile([P, 512], f32, tag="bc", name=f"bc{j}")
                    nc.tensor.matmul(bc[:], sel_hi[:, j, :], rc_hi_b[:],
                                     start=True, stop=True)
                    rcb = dvec.tile([P, 512], bf16, tag="rcb", name=f"rcb{j}")
                    nc.scalar.activation(rcb[:], bc[:], AF.Copy)
                    nc.vector.tensor_tensor(osb[:, j, :], osb[:, j, :], rcb[:],
                                            ALU.mult)
                for ts in range(NTC):
                    xres = xres_t[ts]
                    for oc in range(2):
                        pj = pjp.tile([P, 512], f32, tag="pj")
                        for j in range(NCT):
                            nc.tensor.matmul(pj[:], osb[:, j, ts * P:(ts + 1) * P],
                                             wp_sb[:, j, oc * 512:(oc + 1) * 512],
                                             start=(j == 0), stop=(j == NCT - 1))
                        if zb_proj:
                            nc.vector.tensor_tensor(
                                res1[:, ts, oc * 512:(oc + 1) * 512], pj[:],
                                xres[:, oc * 512:(oc + 1) * 512], ALU.add)
                        else:
                            t1 = dvec.tile([P, 512], f32, tag="t1")
                            nc.vector.tensor_tensor(
                                t1[:], pj[:],
                                xres[:, oc * 512:(oc + 1) * 512], ALU.add)
                            nc.vector.tensor_tensor(
                                res1[:, ts, oc * 512:(oc + 1) * 512], t1[:],
                                bp_bc[:, oc * 512:(oc + 1) * 512], ALU.add)
                    # LN2 + transpose for this token tile, hidden under the
                    # proj matmuls of the next one
                    stats2 = st2.tile([P, 2, 6], f32, tag="bst2")
                    for g in range(2):
                        nc.vector.bn_stats(stats2[:, g, :],
                                           res1[:, ts, g * 512:(g + 1) * 512])
                    mv2 = st2.tile([P, 2], f32, tag="mv2")
                    nc.vector.bn_aggr(mv2[:], stats2[:])
                    sdv2 = st2.tile([P, 1], f32, tag="sdv2")
                    nc.scalar.activation(sdv2[:], mv2[:, 1:2], AF.Sqrt, bias=eps_t[:])
                    rs2 = st2.tile([P, 1], f32, tag="rs2")
                    nc.vector.reciprocal(rs2[:], sdv2[:])
                    nmu = st2.tile([P, 1], f32, tag="nmu")
                    nc.vector.tensor_tensor(nmu[:], mv2[:, 0:1], rs2[:], ALU.mult)
                    nc.vector.tensor_scalar(nmu[:], nmu[:], -1.0, None, ALU.mult)
                    h2 = h2p.tile([P, DIM], bf16, tag="h2")
                    nc.scalar.activation(h2[:], res1[:, ts, :], AF.Identity,
                                         bias=nmu[:], scale=rs2[:])
                    for ct in range(NCT):
                        tp = tps.tile([P, P], bf16, tag="tp")
                        nc.tensor.transpose(tp[:], h2[:, ct * P:(ct + 1) * P], idb[:])
                        if ct % 2 == 0:
                            nc.vector.tensor_copy(
                                h2t[:, ct, ts * P:(ts + 1) * P], tp[:])
                        else:
                            nc.scalar.activation(
                                h2t[:, ct, ts * P:(ts + 1) * P], tp[:], AF.Copy)

            atn.release()
            wpx.release()

            # rest of w1 (space freed by attention buffers)
            w1b = tc.alloc_tile_pool(name="w1b", bufs=1)
            w1_sb_b = w1b.tile([P, 24, NCT, P], bf16)
            for og in range(6):
                nc.sync.dma_start(
                    w1_sb_b[:, og * 4:(og + 1) * 4, :, :],
                    w1h_d[8 + og * 4:8 + (og + 1) * 4].rearrange(
                        "a p b c -> p a b c"))

            # ---------------- Phase E: LN2 + MLP ----------------
            with tc.tile_pool(name="h3p", bufs=1) as h3p, \
                 tc.tile_pool(name="w2p", bufs=6) as w2p, \
                 tc.tile_pool(name="mlv", bufs=2) as mlv, \
                 tc.tile_pool(name="f1s", bufs=2, space="PSUM") as f1s, \
                 tc.tile_pool(name="f2s", bufs=4, space="PSUM") as f2s:
                h3t = h3p.tile([P, HIDDEN // P, 512], bf16)
                if zb_fc2:
                    resb = res1
                else:
                    resb = h3p.tile([P, NTC, DIM], f32)
                    for ts in range(NTC):
                        nc.gpsimd.tensor_tensor(resb[:, ts, :], res1[:, ts, :],
                                                b2_bc[:], ALU.add)

                # fc1 + gelu -> h3t (hidden-major)
                for ot in range(HIDDEN // P):
                    w1_sb = w1_pool[0][1] if ot < 8 else w1_sb_b
                    oti = ot if ot < 8 else ot - 8
                    fp = f1s.tile([P, 512], f32, tag="f1")
                    for ct in range(NCT):
                        nc.tensor.matmul(fp[:], w1_sb[:, oti, ct, :], h2t[:, ct, :],
                                         start=(ct == 0), stop=(ct == NCT - 1))
                    nc.scalar.activation(h3t[:, ot, :], fp[:], AF.Gelu,
                                         bias=b1_pp[:, ot:ot + 1])

                # fc2 + bias + residual -> out
                for oc in range(2):
                    f2t = [f2s.tile([P, 512], f32, tag="f2", name=f"f2_{oc}_{i}")
                           for i in range(NTC)]
                    for cg in range(HIDDEN // P // 2):
                        w2t = w2p.tile([P, 2, 512], bf16, tag="w2t")
                        nc.scalar.dma_start(
                            w2t[:], w2T_d[cg * 256:(cg + 1) * 256,
                                          oc * 512:(oc + 1) * 512]
                            .rearrange("(a p) o -> p a o", p=P))
                        for ci in range(2):
                            ct = cg * 2 + ci
                            for ts in range(NTC):
                                nc.tensor.matmul(f2t[ts][:],
                                                 h3t[:, ct, ts * P:(ts + 1) * P],
                                                 w2t[:, ci, :], start=(ct == 0),
                                                 stop=(ct == HIDDEN // P - 1))
                    for ts in range(NTC):
                        t2 = mlv.tile([P, 512], f32, tag="t22")
                        nc.vector.tensor_tensor(
                            t2[:], f2t[ts][:],
                            resb[:, ts, oc * 512:(oc + 1) * 512], ALU.add)
                        nc.gpsimd.dma_start(
                            out_d[ts * P:(ts + 1) * P, oc * 512:(oc + 1) * 512],
                            t2[:])

            w1_pool[0][0].release()
            w1b.release()

    nc.compile()
    return nc


def _get_program(zb_proj, zb_fc2):
    global _PROG
    key = (zb_proj, zb_fc2)
    if _PROG is None or _PROG[0] != key:
        _PROG = (key, _build_program(zb_proj, zb_fc2))
    return _PROG[1]


def _pack_cols(wT):
    """[C, O] -> [O//128, 128(p), C//128(k), 128(o)] so each o-tile DMA is contiguous."""
    C, O = wT.shape
    # out[ot, p, k, o] = wT[k*128+p, ot*128+o]
    return np.ascontiguousarray(
        wT.reshape(C // P, P, O // P, P).transpose(2, 1, 0, 3))


def _host_prep(x, ln1_g, ln1_b, w_qkv, w_proj, b_proj, ln2_g, ln2_b,
               w_fc1, b_fc1, w_fc2, b_fc2):
    """Per-core input dicts. Pure layout/weight-folding work (no activation math)."""
    f = np.float32
    bf = ml_dtypes.bfloat16
    x = np.asarray(x, f)
    g1 = np.asarray(ln1_g, f); b1 = np.asarray(ln1_b, f)
    g2 = np.asarray(ln2_g, f); b2 = np.asarray(ln2_b, f)
    w_qkv = np.asarray(w_qkv, f); w_proj = np.asarray(w_proj, f)
    w_fc1 = np.asarray(w_fc1, f); w_fc2 = np.asarray(w_fc2, f)
    b_proj = np.asarray(b_proj, f); b_fc1 = np.asarray(b_fc1, f)
    b_fc2 = np.asarray(b_fc2, f)

    wq, wk, wv = w_qkv[0:DIM], w_qkv[DIM:2 * DIM], w_qkv[2 * DIM:3 * DIM]
    wqg = (wq * g1[None, :]).T   # [C(f), O] feature-major, LN gain folded
    wkg = (wk * g1[None, :]).T
    wvg = (wv * g1[None, :]).T
    bq = wq @ b1; bk = wk @ b1; bv = wv @ b1

    wkq = np.concatenate([_pack_cols(wkg), _pack_cols(wqg)], axis=3)
    wkqc = np.stack([
        np.concatenate([wkg.sum(axis=0).reshape(NCT, P),
                        wqg.sum(axis=0).reshape(NCT, P)], axis=1),
        np.concatenate([bk.reshape(NCT, P), bq.reshape(NCT, P)], axis=1),
    ], axis=0)  # [2, 8, 256]

    sel = np.zeros((16, NCT, P), f)
    for j in range(NCT):
        sel[2 * j, j, 0:64] = 1.0
        sel[2 * j + 1, j, 64:128] = 1.0

    bp_total = b_proj + w_proj @ bv
    shared = {
        "wv_d": np.ascontiguousarray(wvg.astype(bf)),
        "wv1_d": np.ascontiguousarray(wvg.sum(axis=0).reshape(1, DIM).astype(bf)),
        "wkq_d": np.ascontiguousarray(wkq.astype(bf)),
        "wkqc_d": np.ascontiguousarray(wkqc.astype(bf)),
        "selb_d": np.ascontiguousarray(sel.astype(bf)),
        "wp_d": np.ascontiguousarray(
            w_proj.T.reshape(NCT, P, DIM).transpose(1, 0, 2).astype(bf)),
        "bpbc_d": np.ascontiguousarray(np.broadcast_to(bp_total, (P, DIM)).astype(f)),
        "w1h_d": np.ascontiguousarray(_pack_cols((w_fc1 * g2[None, :]).T).astype(bf)),
        "b1pp_d": np.ascontiguousarray(
            (b_fc1 + w_fc1 @ b2).reshape(HIDDEN // P, P).T.astype(f)),
        "w2T_d": np.ascontiguousarray(w_fc2.T.astype(bf)),
        "b2bc_d": np.ascontiguousarray(np.broadcast_to(b_fc2, (P, DIM)).astype(f)),
    }
    in_maps = []
    for core in range(8):
        b, q = core // 4, core % 4
        xroll = np.roll(x[b], -CH * q, axis=0)
        m = dict(shared)
        m["xtm_d"] = np.ascontiguousarray(xroll)
        xb = xroll.astype(bf)
        m["xsb_d"] = np.ascontiguousarray(xb)
        m["xTb_d"] = np.ascontiguousarray(xb.T)
        in_maps.append(m)
    return in_maps


def kernel(**inputs) -> np.ndarray:
    _setup_env()
    from concourse import bass_utils

    in_maps = _host_prep(**inputs)
    zb_proj = not np.any(in_maps[0]["bpbc_d"])
    zb_fc2 = not np.any(in_maps[0]["b2bc_d"])
    nc = _get_program(zb_proj, zb_fc2)
    run_kwargs = {}
    if os.environ.get("BASS_PROFILE"):
        import tempfile
        run_kwargs = dict(trace=True, tmpdir=tempfile.mkdtemp(prefix="blk_prof"))
    res = bass_utils.run_bass_kernel_spmd(nc, in_maps, core_ids=list(range(8)),
                                          **run_kwargs)
    kernel.last_result = res
    out = np.empty((2, T, DIM), np.float32)
    for core in range(8):
        b, q = core // 4, core % 4
        out[b, CH * q:CH * (q + 1), :] = res.results[core]["out_d"]
    return out


# revision 25
# speedup vs baseline: 1.0138x; 1.0116x over previous
"""Trainium2 Bass kernel for a pre-norm transformer block (nn_Block_74766790689102).

v2 strategy (8 NeuronCores, zero-communication SPMD):
  core c handles batch b=c//4, query chunk q=c%4 (512 of 2048 tokens); inputs
  are host-rotated so each core's chunk sits at token positions 0:512 -> one
  identical SPMD program for all 8 cores. Each core redundantly computes K/V
  for its whole batch (attention needs all keys).

  Changes vs v1 (716us):
  - bf16 operands for every matmul (fp32 PSUM accumulation): halves weight
    DMA and enables fast weight loads on the PE.
  - Normalized x is never materialized. QKV matmuls consume raw x^T plus a
    rank-2 correction row ((-mu, sdv) x (col-sums, bias)) folded into the
    contraction, then a per-token rstd scale on the outputs. Kills the
    gpsimd/vector normalize traffic and one full 8MB x reload.
  - K, exp'd scores, attention outputs, res1 all stay in SBUF (no DRAM
    roundtrips).
  - K production for head-pair j+1, scores/exp for j, and PV for j-1 are
    interleaved in one PE stream so the ~134us of scalar-engine exp hides
    behind PE work.
  - Softmax denominators collect into one [16,512] tile; a single
    reciprocal_approx_fast + 8 PE broadcasts replace 16 serial [1,512] DVE
    reciprocals (was 52us).
  - V/proj biases fold into the proj bias on the host (softmax rows sum to 1).
  - w1 is fully resident in SBUF before FC1 starts; w2 streams as bf16.
"""

import os
import sys
import types

import numpy as np
import ml_dtypes

DIM = 1024
HEADS = 16
HD = 64
HIDDEN = 4096
T = 2048          # tokens per batch
CH = 512          # chunk tokens per core
SCALE = HD ** -0.5
EPS = 1e-5
NCT = DIM // 128  # 8 feature tiles
NTC = T // 512    # 4 token chunks
NTT = T // 128    # 16 token tiles
P = 128

_ENV_READY = False
_PROG = None


def _setup_env():
    global _ENV_READY
    if _ENV_READY:
        return
    if "/opt/trn_rl_repo" not in sys.path:
        sys.path.insert(0, "/opt/trn_rl_repo")
    # NTFF profile hook shim (the RL container's antenv lacks axon_hooks).
    try:
        import antenv
        if "antenv.axon_hooks" not in sys.modules:
            mod = types.ModuleType("antenv.axon_hooks")
            mod._hook = None
            mod.set_axon_ntff_profile_hook = lambda h: setattr(mod, "_hook", h)
            mod.get_axon_ntff_profile_hook = lambda: mod._hook
            sys.modules["antenv.axon_hooks"] = mod
            antenv.axon_hooks = mod
        if os.environ.get("BASS_PROFILE"):
            from trn_agent_boot.trn_boot import _ntff_profile_via_ctypes
            sys.modules["antenv.axon_hooks"].set_axon_ntff_profile_hook(
                _ntff_profile_via_ctypes("/opt/axon/libaxon_pjrt.so"))
    except Exception:
        pass
    _ENV_READY = True


def _build_program(zb_proj=False, zb_fc2=False):
    """Build + compile the single-core Bass program (same for all 8 cores).

    zb_proj/zb_fc2: host-verified all-zero proj/fc2 biases let the epilogues
    skip the dead bias adds (general path kept as fallback).
    """
    _setup_env()
    import concourse.bacc as bacc
    import concourse.tile as tile
    import concourse.mybir as mybir
    from concourse.masks import make_identity

    dt = mybir.dt
    AF = mybir.ActivationFunctionType
    ALU = mybir.AluOpType
    f32, bf16 = dt.float32, dt.bfloat16

    nc = bacc.Bacc("TRN2", target_bir_lowering=False, debug=False, num_devices=8)

    # ---- I/O ----
    xtm_d = nc.dram_tensor("xtm_d", [T, DIM], f32, kind="ExternalInput").ap()
    xsb_d = nc.dram_tensor("xsb_d", [T, DIM], bf16, kind="ExternalInput").ap()
    xTb_d = nc.dram_tensor("xTb_d", [DIM, T], bf16, kind="ExternalInput").ap()
    wv_d = nc.dram_tensor("wv_d", [DIM, DIM], bf16, kind="ExternalInput").ap()
    wv1_d = nc.dram_tensor("wv1_d", [1, DIM], bf16, kind="ExternalInput").ap()
    wkq_d = nc.dram_tensor("wkq_d", [NCT, P, NCT, 256], bf16, kind="ExternalInput").ap()
    wkqc_d = nc.dram_tensor("wkqc_d", [2, NCT, 256], bf16, kind="ExternalInput").ap()
    selb_d = nc.dram_tensor("selb_d", [16, NCT, P], bf16, kind="ExternalInput").ap()
    wp_d = nc.dram_tensor("wp_d", [P, NCT, DIM], bf16, kind="ExternalInput").ap()
    bpbc_d = nc.dram_tensor("bpbc_d", [P, DIM], f32, kind="ExternalInput").ap()
    w1h_d = nc.dram_tensor("w1h_d", [HIDDEN // P, P, NCT, P], bf16, kind="ExternalInput").ap()
    b1pp_d = nc.dram_tensor("b1pp_d", [P, HIDDEN // P], f32, kind="ExternalInput").ap()
    w2T_d = nc.dram_tensor("w2T_d", [HIDDEN, DIM], bf16, kind="ExternalInput").ap()
    b2bc_d = nc.dram_tensor("b2bc_d", [P, DIM], f32, kind="ExternalInput").ap()
    out_d = nc.dram_tensor("out_d", [CH, DIM], f32, kind="ExternalOutput").ap()

    with tile.TileContext(nc) as tc:
        with tc.tile_pool(name="cst", bufs=1) as cst, \
             tc.tile_pool(name="resp", bufs=1) as resp:

            # ---------------- constants ----------------
            idf = cst.tile([P, P], f32)
            make_identity(nc, idf[:])
            idb = cst.tile([P, P], bf16)
            nc.vector.tensor_copy(idb[:], idf[:])
            ones1b = cst.tile([1, P], bf16)
            nc.vector.memset(ones1b[:], 1.0)
            eps_t = cst.tile([P, 1], f32)
            nc.vector.memset(eps_t[:], EPS)
            warm = cst.tile([P, 1], f32)
            # head-pair selection matrices for the denominator broadcast,
            # split lo/hi so each half is a base-partition-0 matmul operand
            sel_lo = cst.tile([8, NCT, P], bf16)
            nc.gpsimd.dma_start(sel_lo[:], selb_d[0:8])
            sel_hi = cst.tile([8, NCT, P], bf16)
            nc.gpsimd.dma_start(sel_hi[:], selb_d[8:16])
            corr2 = cst.tile([2, T], bf16)     # rows: -mu, sdv (per token)
            rsr = cst.tile([1, T], bf16)       # rstd row (per token)
            rs_cols = cst.tile([P, NTT], f32)  # rstd, token-partition layout
            dn_lo = cst.tile([8, 512], f32)    # denominators, heads 0-7
            dn_hi = cst.tile([8, 512], f32)    # denominators, heads 8-15
            rc_lo_f = cst.tile([8, 512], f32)
            rc_hi_f = cst.tile([8, 512], f32)
            rc_lo_b = cst.tile([8, 512], bf16)
            rc_hi_b = cst.tile([8, 512], bf16)
            b1_pp = cst.tile([P, HIDDEN // P], f32)
            nc.gpsimd.dma_start(b1_pp[:], b1pp_d[:])

            res1 = resp.tile([P, NTC, DIM], f32)   # attn residual stream
            h2t = resp.tile([P, NCT, 512], bf16)   # LN2(res1)^T for FC1
            b2_bc = resp.tile([P, DIM], f32)
            nc.gpsimd.dma_start(b2_bc[:], b2bc_d[:])

            # attention-lifetime SBUF (released after proj)
            atn = tc.alloc_tile_pool(name="atn", bufs=1)
            vsb = atn.tile([P, NTT, HEADS, 65], bf16)   # V-hat + ones col 64
            q_sb = atn.tile([P, NCT, 512], bf16)
            k_sb = atn.tile([P, NCT, T], bf16)
            osb = atn.tile([P, NCT, 512], bf16)         # per-pair attn out
            rb_sb = atn.tile([P, NTC, 512], bf16)       # rstd broadcast

            nc.vector.memset(
                vsb[:, :, :, 64:65].rearrange("p a b c -> p (a b c)"), 1.0)

            # raw x^T + K/Q weights (released before FC1 weight prefetch)
            xTp = tc.alloc_tile_pool(name="xTp", bufs=1, side="right")
            xT_sb = xTp.tile([P, NCT, T], bf16)
            kqw = tc.alloc_tile_pool(name="kqw", bufs=2, side="right")
            wkqc_sb = kqw.tile([2, NCT, 256], bf16, tag="wkqc", bufs=1)
            nc.gpsimd.dma_start(wkqc_sb[:], wkqc_d[:])

            kqp = tc.alloc_tile_pool(name="kqp", bufs=1, space="PSUM")
            wkq_tiles = {}

            def load_wkq(j):
                w = kqw.tile([P, NCT, 256], bf16, tag="wkq", name=f"wkq{j}")
                nc.sync.dma_start(w[:], wkq_d[j])
                wkq_tiles[j] = w

            def emit_k_chunk(j, tcn):
                w = wkq_tiles[j]
                kp = kqp.tile([P, 512], f32, tag="kq", name=f"kp{j}_{tcn}")
                sl = slice(tcn * 512, (tcn + 1) * 512)
                for k in range(NCT):
                    nc.tensor.matmul(kp[:], w[:, k, 0:P], xT_sb[:, k, sl],
                                     start=(k == 0), stop=False)
                nc.tensor.matmul(kp[:], wkqc_sb[:, j, 0:P], corr2[0:2, sl],
                                 start=False, stop=True)
                nc.vector.tensor_tensor(k_sb[:, j, sl], kp[:], rb_sb[:, tcn, :],
                                        ALU.mult)

            def emit_q(j, pop=True):
                w = wkq_tiles.pop(j) if pop else wkq_tiles[j]
                qp = kqp.tile([P, 512], f32, tag="kq", name=f"qp{j}")
                for k in range(NCT):
                    nc.tensor.matmul(qp[:], w[:, k, P:256], xT_sb[:, k, 0:512],
                                     start=(k == 0), stop=False)
                nc.tensor.matmul(qp[:], wkqc_sb[:, j, P:256], corr2[0:2, 0:512],
                                 start=False, stop=True)
                nc.vector.tensor_tensor(q_sb[:, j, :], qp[:], rb_sb[:, 0, :],
                                        ALU.mult)

            # ---------------- Phase A: LN1 stats + V-hat ----------------
            with tc.tile_pool(name="xsp", bufs=4) as xsp, \
                 tc.tile_pool(name="stp", bufs=2) as stp, \
                 tc.tile_pool(name="wvp", bufs=1) as wvp, \
                 tc.tile_pool(name="aps", bufs=1, space="PSUM") as aps, \
                 tc.tile_pool(name="rbp", bufs=1, space="PSUM") as rbp, \
                 tc.tile_pool(name="vps", bufs=4, space="PSUM") as vps:
                wv_sb = wvp.tile([P, NCT, DIM], bf16)
                wv1_sb = wvp.tile([1, DIM], bf16)

                def nc_wv1(oc):
                    return wv1_sb[0:1, oc * 512:(oc + 1) * 512]
                # stats tiles first (tiny, feed the corr rows), then wv (sync)
                # and x^T (scalar) slices dispatched on separate queues so the
                # first V-hat k-chain starts within ~2us
                xs_pre = {}
                for s in range(3):
                    xs = xsp.tile([P, DIM], bf16, tag="xs", name=f"xs{s}")
                    if s == 0:
                        for g in range(2):
                            nc.sync.dma_start(
                                xs[:, g * 512:(g + 1) * 512],
                                xsb_d[0:P, g * 512:(g + 1) * 512])
                    else:
                        nc.sync.dma_start(xs[:], xsb_d[s * P:(s + 1) * P, :])
                    xs_pre[s] = xs
                # first two k-slices split column-wise so they spread over
                # several DMA queues (a whole 512KB slice on one queue takes
                # ~12us and gates the first V-hat matmul)
                for k in range(2):
                    for c in range(4):
                        nc.sync.dma_start(
                            wv_sb[:, k, c * 256:(c + 1) * 256],
                            wv_d[k * P:(k + 1) * P, c * 256:(c + 1) * 256])
                        nc.scalar.dma_start(
                            xT_sb[:, k, c * 512:(c + 1) * 512],
                            xTb_d[k * P:(k + 1) * P, c * 512:(c + 1) * 512])
                for k in range(2, 4):
                    for c in range(2):
                        nc.sync.dma_start(
                            wv_sb[:, k, c * 512:(c + 1) * 512],
                            wv_d[k * P:(k + 1) * P, c * 512:(c + 1) * 512])
                        nc.scalar.dma_start(
                            xT_sb[:, k, c * 1024:(c + 1) * 1024],
                            xTb_d[k * P:(k + 1) * P, c * 1024:(c + 1) * 1024])
                for k in range(4, NCT):
                    nc.sync.dma_start(wv_sb[:, k, :],
                                      wv_d[k * P:(k + 1) * P, :])
                    nc.scalar.dma_start(xT_sb[:, k, :],
                                        xTb_d[k * P:(k + 1) * P, :])
                nc.sync.dma_start(wv1_sb[:], wv1_d[:])
                nc.scalar.activation(warm[:], eps_t[:], AF.Sqrt)
                load_wkq(0)
                load_wkq(1)
                for s in range(NTT):
                    if s in xs_pre:
                        xs = xs_pre.pop(s)
                    else:
                        xs = xsp.tile([P, DIM], bf16, tag="xs", name=f"xs{s}")
                        nc.sync.dma_start(xs[:], xsb_d[s * P:(s + 1) * P, :])
                    stats = stp.tile([P, 2, 6], f32, tag="bst")
                    for g in range(2):
                        nc.vector.bn_stats(stats[:, g, :], xs[:, g * 512:(g + 1) * 512])
                    mv = stp.tile([P, 2], f32, tag="mv")
                    nc.vector.bn_aggr(mv[:], stats[:])
                    stk = stp.tile([P, 2], f32, tag="stk")
                    nc.vector.tensor_scalar(stk[:, 0:1], mv[:, 0:1], -1.0, None, ALU.mult)
                    nc.scalar.activation(stk[:, 1:2], mv[:, 1:2], AF.Sqrt, bias=eps_t[:])
                    nc.vector.reciprocal(rs_cols[:, s:s + 1], stk[:, 1:2])
                    # V-hat k-chains first: they only need x/wv tiles, so the
                    # PE is not gated on the stats chain above.
                    vp2 = []
                    for oc in range(2):
                        vp = vps.tile([P, 512], f32, tag="vp")
                        for k in range(NCT):
                            nc.tensor.matmul(vp[:], xT_sb[:, k, s * P:(s + 1) * P],
                                             wv_sb[:, k, oc * 512:(oc + 1) * 512],
                                             start=(k == 0), stop=False)
                        vp2.append(vp)
                    pst = aps.tile([2, P], f32, tag="pst")
                    nc.tensor.transpose(pst[:], stk[:], idf[:])
                    nc.vector.tensor_copy(corr2[:, s * P:(s + 1) * P], pst[:])
                    pst1 = aps.tile([1, P], f32, tag="pst1")
                    nc.tensor.transpose(pst1[:], rs_cols[:, s:s + 1], idf[:])
                    nc.vector.tensor_copy(rsr[:, s * P:(s + 1) * P], pst1[:])
                    for oc in range(2):
                        vp = vp2[oc]
                        nc.tensor.matmul(vp[:], corr2[0:1, s * P:(s + 1) * P],
                                         nc_wv1(oc),
                                         start=False, stop=True)
                        nc.vector.tensor_scalar(
                            vsb[:, s, oc * 8:(oc + 1) * 8, 0:64],
                            vp[:].rearrange("p (h d) -> p h d", d=64),
                            rs_cols[:, s:s + 1], None, ALU.mult)
                    if s % 4 == 3:
                        tcn = s // 4
                        rb = rbp.tile([P, 512], f32, tag="rb")
                        nc.tensor.matmul(rb[:], ones1b[:],
                                         rsr[0:1, tcn * 512:(tcn + 1) * 512],
                                         start=True, stop=True)
                        nc.vector.tensor_copy(rb_sb[:, tcn, :], rb[:])
                        # hide the first two K head-tiles + Q(0) under V-hat
                        emit_k_chunk(0, tcn)
                        emit_k_chunk(1, tcn)
                        if s == 7:
                            emit_q(0, pop=False)
                        if s == NTT - 1:
                            wkq_tiles.pop(0)
                            load_wkq(2)

            # ---------------- Phase B+C: scores/exp + PV + remaining K/Q ----------------
            psb_slots = {}
            w1_pool = []   # filled mid-phase once xT frees

            with tc.tile_pool(name="psbp", bufs=8) as psbp, \
                 tc.tile_pool(name="tvec", bufs=2) as tvec, \
                 tc.tile_pool(name="spp", bufs=2, space="PSUM") as spp, \
                 tc.tile_pool(name="pvp", bufs=3, space="PSUM") as pvp:

                def emit_s(j, kt):
                    sp = spp.tile([P, 2, 512], f32, tag="sp", name=f"sp{j}_{kt}")
                    ksl = slice(kt * P, (kt + 1) * P)
                    nc.tensor.matmul(sp[:, 0, :], k_sb[0:64, j, ksl],
                                     q_sb[0:64, j, :], start=True, stop=True)
                    nc.tensor.matmul(sp[:, 1, :], k_sb[64:128, j, ksl],
                                     q_sb[64:128, j, :], start=True, stop=True)
                    slot = psbp.tile([P, 2, 512], bf16, tag="psb",
                                     name=f"psb{j}_{kt}")
                    nc.scalar.activation(slot[:], sp[:], AF.Exp, scale=SCALE)
                    psb_slots[(j, kt)] = slot

                def emit_pv_kt(j, kt, pvA, pvB):
                    slot = psb_slots.pop((j, kt))
                    nc.tensor.matmul(pvA[:], vsb[:, kt, 2 * j, :], slot[:, 0, :],
                                     start=(kt == 0), stop=(kt == NTT - 1))
                    nc.tensor.matmul(pvB[:], vsb[:, kt, 2 * j + 1, :], slot[:, 1, :],
                                     start=(kt == 0), stop=(kt == NTT - 1))

                def emit_tail(h, pv):
                    dtmp = tvec.tile([65, 512], f32, tag="dtmp", name=f"dt{h}",
                                     bufs=2)
                    nc.scalar.activation(dtmp[64:65, :], pv[64:65, :], AF.Copy)
                    dnt = dn_lo if h < 8 else dn_hi
                    nc.gpsimd.dma_start(dnt[h % 8:h % 8 + 1, :], dtmp[64:65, :])
                    if h % 2 == 0:
                        nc.vector.tensor_copy(osb[0:64, h // 2, :], pv[0:64, :])
                    else:
                        ot = tvec.tile([64, 512], bf16, tag="ot", name=f"ot{h}")
                        nc.vector.tensor_copy(ot[:], pv[0:64, :])
                        nc.gpsimd.dma_start(osb[64:128, h // 2, :], ot[:])

                LAG = 4

                def emit_osb_scale(j, sel_t, rc_b, pool, tag):
                    bc = pool.tile([P, 512], f32, tag=tag, name=f"bcs{j}")
                    nc.tensor.matmul(bc[:], sel_t[:, j, :], rc_b[:],
                                     start=True, stop=True)
                    rcb = tvec.tile([P, 512], bf16, tag="rcb", name=f"rcb{j}",
                                    bufs=2)
                    nc.vector.tensor_copy(rcb[:], bc[:])
                    nc.vector.tensor_tensor(osb[:, j, :], osb[:, j, :], rcb[:],
                                            ALU.mult)

                for j in range(NCT):
                    if j == NCT - 1:
                        # x^T and K/Q weights are dead: free them; prefetch
                        # proj weights and the first w1 slice under the last
                        # scores/PV round.
                        kqw.release()
                        xTp.release()
                        w1a = tc.alloc_tile_pool(name="w1a", bufs=1,
                                                 side="right")
                        w1_sb_a = w1a.tile([P, 8, NCT, P], bf16)
                        for og in range(2):
                            nc.sync.dma_start(
                                w1_sb_a[:, og * 4:(og + 1) * 4, :, :],
                                w1h_d[og * 4:(og + 1) * 4].rearrange(
                                    "a p b c -> p a b c"))
                        w1_pool.append((w1a, w1_sb_a))
                        wpx = tc.alloc_tile_pool(name="wpx", bufs=1,
                                                 side="right")
                        wp_sb = wpx.tile([P, NCT, DIM], bf16)
                        nc.sync.dma_start(wp_sb[:], wp_d[:])
                        bp_bc = wpx.tile([P, DIM], f32)
                        nc.sync.dma_start(bp_bc[:], bpbc_d[:])
                    if j == NCT - 1:
                        # pairs 0-3 finished long ago: normalize their attn
                        # outputs now, hidden under the last scores round
                        nc.vector.reciprocal_approx_fast(rc_lo_f[:], dn_lo[:])
                        nc.vector.tensor_copy(rc_lo_b[:], rc_lo_f[:])
                        for jj in range(4):
                            emit_osb_scale(jj, sel_lo, rc_lo_b, kqp, "kq")
                    pvA = pvp.tile([65, 512], f32, tag="pv", name=f"pvA{j}")
                    pvB = pvp.tile([65, 512], f32, tag="pv", name=f"pvB{j}")
                    for kt in range(NTT):
                        if not (j == NCT - 1 and kt < 4):
                            emit_s(j, kt)
                        if kt == 1 and j + 2 < NCT:
                            emit_k_chunk(j + 2, 0)
                        elif kt == 5 and j + 2 < NCT:
                            emit_k_chunk(j + 2, 1)
                        elif kt == 9 and 2 <= j + 1 < NCT:
                            emit_k_chunk(j + 1, 2)
                        elif kt == 13 and 2 <= j + 1 < NCT:
                            emit_k_chunk(j + 1, 3)
                        elif kt == 14 and j + 1 < NCT - 1:
                            emit_q(j + 1)
                            if j + 3 < NCT:
                                load_wkq(j + 3)
                        if j == NCT - 2:
                            # front-load the last pair's Q and first scores so
                            # its exps start a full iteration early
                            if kt == 1:
                                emit_q(NCT - 1, pop=False)
                            elif kt in (3, 7, 11, 15):
                                emit_s(NCT - 1, (kt - 3) // 4)
                        if kt >= LAG:
                            emit_pv_kt(j, kt - LAG, pvA, pvB)
                    for kt in range(NTT - LAG, NTT):
                        emit_pv_kt(j, kt, pvA, pvB)
                    emit_tail(2 * j, pvA)
                    emit_tail(2 * j + 1, pvB)

            # ---------------- Phase D: normalize o, proj + residual ----------------
            kqp.release()
            with tc.tile_pool(name="dvec", bufs=2) as dvec, \
                 tc.tile_pool(name="xrp", bufs=4) as xrp, \
                 tc.tile_pool(name="st2", bufs=2) as st2, \
                 tc.tile_pool(name="h2p", bufs=2) as h2p, \
                 tc.tile_pool(name="bcp", bufs=2, space="PSUM") as bcp, \
                 tc.tile_pool(name="tps", bufs=2, space="PSUM") as tps, \
                 tc.tile_pool(name="pjp", bufs=2, space="PSUM") as pjp:
                xres_t = []
                for ts in range(NTC):
                    xres = xrp.tile([P, DIM], f32, tag="xres", name=f"xr{ts}")
                    nc.sync.dma_start(xres[:], xtm_d[ts * P:(ts + 1) * P, :])
                    xres_t.append(xres)
                nc.vector.reciprocal_approx_fast(rc_hi_f[:], dn_hi[:])
                nc.vector.tensor_copy(rc_hi_b[:], rc_hi_f[:])
                for j in range(4, NCT):
                    bc = bcp.tile([P, 512], f32, tag="bc", name=f"bc{j}")
                    nc.tensor.matmul(bc[:], sel_hi[:, j, :], rc_hi_b[:],
                                     start=True, stop=True)
                    rcb = dvec.tile([P, 512], bf16, tag="rcb", name=f"rcb{j}")
                    nc.scalar.activation(rcb[:], bc[:], AF.Copy)
                    nc.vector.tensor_tensor(osb[:, j, :], osb[:, j, :], rcb[:],
                                            ALU.mult)
                for ts in range(NTC):
                    xres = xres_t[ts]
                    for oc in range(2):
                        pj = pjp.tile([P, 512], f32, tag="pj")
                        for j in range(NCT):
                            nc.tensor.matmul(pj[:], osb[:, j, ts * P:(ts + 1) * P],
                                             wp_sb[:, j, oc * 512:(oc + 1) * 512],
                                             start=(j == 0), stop=(j == NCT - 1))
                        if zb_proj:
                            nc.vector.tensor_tensor(
                                res1[:, ts, oc * 512:(oc + 1) * 512], pj[:],
                                xres[:, oc * 512:(oc + 1) * 512], ALU.add)
                        else:
                            t1 = dvec.tile([P, 512], f32, tag="t1")
                            nc.vector.tensor_tensor(
                                t1[:], pj[:],
                                xres[:, oc * 512:(oc + 1) * 512], ALU.add)
                            nc.vector.tensor_tensor(
                                res1[:, ts, oc * 512:(oc + 1) * 512], t1[:],
                                bp_bc[:, oc * 512:(oc + 1) * 512], ALU.add)
                    # LN2 + transpose for this token tile, hidden under the
                    # proj matmuls of the next one
                    stats2 = st2.tile([P, 2, 6], f32, tag="bst2")
                    for g in range(2):
                        nc.vector.bn_stats(stats2[:, g, :],
                                           res1[:, ts, g * 512:(g + 1) * 512])
                    mv2 = st2.tile([P, 2], f32, tag="mv2")
                    nc.vector.bn_aggr(mv2[:], stats2[:])
                    sdv2 = st2.tile([P, 1], f32, tag="sdv2")
                    nc.scalar.activation(sdv2[:], mv2[:, 1:2], AF.Sqrt, bias=eps_t[:])
                    rs2 = st2.tile([P, 1], f32, tag="rs2")
                    nc.vector.reciprocal(rs2[:], sdv2[:])
                    nmu = st2.tile([P, 1], f32, tag="nmu")
                    nc.vector.tensor_tensor(nmu[:], mv2[:, 0:1], rs2[:], ALU.mult)
                    nc.vector.tensor_scalar(nmu[:], nmu[:], -1.0, None, ALU.mult)
                    h2 = h2p.tile([P, DIM], bf16, tag="h2")
                    nc.scalar.activation(h2[:], res1[:, ts, :], AF.Identity,
                                         bias=nmu[:], scale=rs2[:])
                    for ct in range(NCT):
                        tp = tps.tile([P, P], bf16, tag="tp")
                        nc.tensor.transpose(tp[:], h2[:, ct * P:(ct + 1) * P], idb[:])
                        if ct % 2 == 0:
                            nc.vector.tensor_copy(
                                h2t[:, ct, ts * P:(ts + 1) * P], tp[:])
                        else:
                            nc.scalar.activation(
                                h2t[:, ct, ts * P:(ts + 1) * P], tp[:], AF.Copy)

            atn.release()
            wpx.release()

            # rest of w1 (space freed by attention buffers)
            w1b = tc.alloc_tile_pool(name="w1b", bufs=1)
            w1_sb_b = w1b.tile([P, 24, NCT, P], bf16)
            for og in range(6):
                nc.sync.dma_start(
                    w1_sb_b[:, og * 4:(og + 1) * 4, :, :],
                    w1h_d[8 + og * 4:8 + (og + 1) * 4].rearrange(
                        "a p b c -> p a b c"))

            # ---------------- Phase E: LN2 + MLP ----------------
            with tc.tile_pool(name="h3p", bufs=1) as h3p, \
                 tc.tile_pool(name="w2p", bufs=6) as w2p, \
                 tc.tile_pool(name="mlv", bufs=2) as mlv, \
                 tc.tile_pool(name="f1s", bufs=2, space="PSUM") as f1s, \
                 tc.tile_pool(name="f2s", bufs=4, space="PSUM") as f2s:
                h3t = h3p.tile([P, HIDDEN // P, 512], bf16)
                if zb_fc2:
                    resb = res1
                else:
                    resb = h3p.tile([P, NTC, DIM], f32)
                    for ts in range(NTC):
                        nc.gpsimd.tensor_tensor(resb[:, ts, :], res1[:, ts, :],
                                                b2_bc[:], ALU.add)

                # fc1 + gelu -> h3t (hidden-major)
                for ot in range(HIDDEN // P):
                    w1_sb = w1_pool[0][1] if ot < 8 else w1_sb_b
                    oti = ot if ot < 8 else ot - 8
                    fp = f1s.tile([P, 512], f32, tag="f1")
                    for ct in range(NCT):
                        nc.tensor.matmul(fp[:], w1_sb[:, oti, ct, :], h2t[:, ct, :],
                                         start=(ct == 0), stop=(ct == NCT - 1))
                    nc.scalar.activation(h3t[:, ot, :], fp[:], AF.Gelu,
                                         bias=b1_pp[:, ot:ot + 1])

                # fc2 + bias + residual -> out
                for oc in range(2):
                    f2t = [f2s.tile([P, 512], f32, tag="f2", name=f"f2_{oc}_{i}")
                           for i in range(NTC)]
                    for cg in range(HIDDEN // P // 2):
                        w2t = w2p.tile([P, 2, 512], bf16, tag="w2t")
                        nc.scalar.dma_start(
                            w2t[:], w2T_d[cg * 256:(cg + 1) * 256,
                                          oc * 512:(oc + 1) * 512]
                            .rearrange("(a p) o -> p a o", p=P))
                        for ci in range(2):
                            ct = cg * 2 + ci
                            for ts in range(NTC):
                                nc.tensor.matmul(f2t[ts][:],
                                                 h3t[:, ct, ts * P:(ts + 1) * P],
                                                 w2t[:, ci, :], start=(ct == 0),
                                                 stop=(ct == HIDDEN // P - 1))
                    for ts in range(NTC):
                        t2 = mlv.tile([P, 512], f32, tag="t22")
                        nc.vector.tensor_tensor(
                            t2[:], f2t[ts][:],
                            resb[:, ts, oc * 512:(oc + 1) * 512], ALU.add)
                        nc.gpsimd.dma_start(
                            out_d[ts * P:(ts + 1) * P, oc * 512:(oc + 1) * 512],
                            t2[:])

            w1_pool[0][0].release()
            w1b.release()

    nc.compile()
    return nc


def _get_program(zb_proj, zb_fc2):
    global _PROG
    key = (zb_proj, zb_fc2)
    if _PROG is None or _PROG[0] != key:
        _PROG = (key, _build_program(zb_proj, zb_fc2))
    return _PROG[1]


def _pack_cols(wT):
    """[C, O] -> [O//128, 128(p), C//128(k), 128(o)] so each o-tile DMA is contiguous."""
    C, O = wT.shape
    # out[ot, p, k, o] = wT[k*128+p, ot*128+o]
    return np.ascontiguousarray(
        wT.reshape(C // P, P, O // P, P).transpose(2, 1, 0, 3))


def _host_prep(x, ln1_g, ln1_b, w_qkv, w_proj, b_proj, ln2_g, ln2_b,
               w_fc1, b_fc1, w_fc2, b_fc2):
    """Per-core input dicts. Pure layout/weight-folding work (no activation math)."""
    f = np.float32
    bf = ml_dtypes.bfloat16
    x = np.asarray(x, f)
    g1 = np.asarray(ln1_g, f); b1 = np.asarray(ln1_b, f)
    g2 = np.asarray(ln2_g, f); b2 = np.asarray(ln2_b, f)
    w_qkv = np.asarray(w_qkv, f); w_proj = np.asarray(w_proj, f)
    w_fc1 = np.asarray(w_fc1, f); w_fc2 = np.asarray(w_fc2, f)
    b_proj = np.asarray(b_proj, f); b_fc1 = np.asarray(b_fc1, f)
    b_fc2 = np.asarray(b_fc2, f)

    wq, wk, wv = w_qkv[0:DIM], w_qkv[DIM:2 * DIM], w_qkv[2 * DIM:3 * DIM]
    wqg = (wq * g1[None, :]).T   # [C(f), O] feature-major, LN gain folded
    wkg = (wk * g1[None, :]).T
    wvg = (wv * g1[None, :]).T
    bq = wq @ b1; bk = wk @ b1; bv = wv @ b1

    wkq = np.concatenate([_pack_cols(wkg), _pack_cols(wqg)], axis=3)
    wkqc = np.stack([
        np.concatenate([wkg.sum(axis=0).reshape(NCT, P),
                        wqg.sum(axis=0).reshape(NCT, P)], axis=1),
        np.concatenate([bk.reshape(NCT, P), bq.reshape(NCT, P)], axis=1),
    ], axis=0)  # [2, 8, 256]

    sel = np.zeros((16, NCT, P), f)
    for j in range(NCT):
        sel[2 * j, j, 0:64] = 1.0
        sel[2 * j + 1, j, 64:128] = 1.0

    bp_total = b_proj + w_proj @ bv
    shared = {
        "wv_d": np.ascontiguousarray(wvg.astype(bf)),
        "wv1_d": np.ascontiguousarray(wvg.sum(axis=0).reshape(1, DIM).astype(bf)),
        "wkq_d": np.ascontiguousarray(wkq.astype(bf)),
        "wkqc_d": np.ascontiguousarray(wkqc.astype(bf)),
        "selb_d": np.ascontiguousarray(sel.astype(bf)),
        "wp_d": np.ascontiguousarray(
            w_proj.T.reshape(NCT, P, DIM).transpose(1, 0, 2).astype(bf)),
        "bpbc_d": np.ascontiguousarray(np.broadcast_to(bp_total, (P, DIM)).astype(f)),
        "w1h_d": np.ascontiguousarray(_pack_cols((w_fc1 * g2[None, :]).T).astype(bf)),
        "b1pp_d": np.ascontiguousarray(
            (b_fc1 + w_fc1 @ b2).reshape(HIDDEN // P, P).T.astype(f)),
        "w2T_d": np.ascontiguousarray(w_fc2.T.astype(bf)),
        "b2bc_d": np.ascontiguousarray(np.broadcast_to(b_fc2, (P, DIM)).astype(f)),
    }
    in_maps = []
    for core in range(8):
        b, q = core // 4, core % 4
        xroll = np.roll(x[b], -CH * q, axis=0)
        m = dict(shared)
        m["xtm_d"] = np.ascontiguousarray(xroll)
        xb = xroll.astype(bf)
        m["xsb_d"] = np.ascontiguousarray(xb)
        m["xTb_d"] = np.ascontiguousarray(xb.T)
        in_maps.append(m)
    return in_maps


def kernel(**inputs) -> np.ndarray:
    _setup_env()
    from concourse import bass_utils

    in_maps = _host_prep(**inputs)
    zb_proj = not np.any(in_maps[0]["bpbc_d"])
    zb_fc2 = not np.any(in_maps[0]["b2bc_d"])
    nc = _get_program(zb_proj, zb_fc2)
    run_kwargs = {}
    if os.environ.get("BASS_PROFILE"):
        import tempfile
        run_kwargs = dict(trace=True, tmpdir=tempfile.mkdtemp(prefix="blk_prof"))
    res = bass_utils.run_bass_kernel_spmd(nc, in_maps, core_ids=list(range(8)),
                                          **run_kwargs)
    kernel.last_result = res
    out = np.empty((2, T, DIM), np.float32)
    for core in range(8):
        b, q = core // 4, core % 4
        out[b, CH * q:CH * (q + 1), :] = res.results[core]["out_d"]
    return out
